# revision 1
# baseline (speedup 1.0000x reference)
"""GAT (3-layer, PyG-style) forward on 8 Trainium2 NeuronCores.

Strategy:
  - Node space padded to 8*PC nodes; core c owns nodes [c*PC, (c+1)*PC).
  - Per layer, a "table" in DRAM holds one 256B row per node:
      [h (64 bf16) | as (4 f32) | ad (4 f32) | pad], where
      as/ad are the per-node attention terms (x @ W_as etc, folded into an
      augmented weight matrix on the host).
  - Edge phase (per core, per 128-node tile): dma_gather the rows of all
    in-neighbours (4 int16-addressable windows of the table), compute
    e = leakyrelu(as_src + ad_dst), ex = exp(e) (no max subtraction: values
    are tiny, softmax is shift-invariant), U = sum ex*h via a halving tree,
    denom = sum ex; self-loop handled densely from the node's own row.
  - out = U/denom (mean over heads for concat=False) + bias (+elu), then
    PE-transpose and AllGather so every core can rebuild the full next-layer
    table.
  - Final: per-tile pooling matmul into PSUM, AllReduce, MLP head on-device.

Host does: graph preprocessing (degree profiles, tile clustering, slot/pad
assignment, int16 index arrays), weight augmentation/transposition, and the
final [1,64] -> [64,1] reshape.
"""

import sys

sys.path.insert(0, "/opt/trn_rl_repo")

import numpy as np
import ml_dtypes

BF16 = ml_dtypes.bfloat16

NEG_SLOPE = 0.2
DUMMY_AS = -30000.0
MAXJ_CALL = 14  # <=1792 idx per dma_gather (ring 2048 w/ 32KB scratch)
NQ = 4         # SWDGE queues


# ----------------------------------------------------------------- host prep

def _prep(x, edge_index, batch, n_graphs):
    """Graph preprocessing. Returns a dict of host arrays + structure."""
    N = x.shape[0]
    NC = 8
    PC = int(np.ceil(N / NC / 128)) * 128          # nodes per core (padded)
    NSTAR = NC * PC
    TILES = PC // 128
    BLKROWS = PC + 1                               # +1 dummy row per core blk
    WIN = 2 * BLKROWS                              # gather window (2 blocks)
    assert WIN <= 32767

    src = edge_index[0].astype(np.int64)
    dst = edge_index[1].astype(np.int64)

    core_of = np.arange(NSTAR) // PC               # orig id -> core
    win_of = (core_of // 2).astype(np.int64)       # orig id -> window

    # per-dst in-degree per window (real edges only; self-loops added densely)
    degw = np.zeros((NSTAR, 4), np.int64)
    np.add.at(degw, (dst, win_of[src]), 1)

    # --- cluster nodes into tiles (per core) by window-degree profile ----
    # rank_of[orig] = position within its core (tile*128 + slot)
    rank_of = np.empty(NSTAR, np.int64)
    tile_K = np.zeros((NC, TILES, 4), np.int64)    # per-core per-tile max deg
    for c in range(NC):
        ids = np.arange(c * PC, (c + 1) * PC)
        prof = degw[ids].astype(np.int64)
        # greedy 4-D bin packing: place big nodes first into the bin where
        # the sum-of-window-maxima grows least
        order0 = np.argsort(-prof.sum(axis=1), kind="stable")
        bins_max = np.zeros((TILES, 4), np.int64)
        bins_cnt = np.zeros(TILES, np.int64)
        assign = np.empty(PC, np.int64)
        slot_in = np.empty(PC, np.int64)
        for j in order0:
            v = prof[j]
            delta = np.maximum(bins_max, v).sum(axis=1) - bins_max.sum(axis=1)
            delta = delta + (bins_cnt >= 128) * (1 << 30)
            b_ = int(np.argmin(delta))
            assign[j] = b_
            slot_in[j] = bins_cnt[b_]
            bins_cnt[b_] += 1
            bins_max[b_] = np.maximum(bins_max[b_], v)
        rank_of[ids] = assign * 128 + slot_in
        tile_K[c] = bins_max

    # order each core's tiles by K-profile (descending total) so that the
    # r-th tile of each core has a similar profile across cores (SPMD
    # uniformity: round r uses K_s(r) = max over cores).
    for c in range(NC):
        tot = tile_K[c].sum(axis=1)
        t_order = np.argsort(-tot, kind="stable")
        # remap ranks: tile t_order[i] becomes tile i
        newpos = np.empty(TILES, np.int64)
        newpos[t_order] = np.arange(TILES)
        ids = np.arange(c * PC, (c + 1) * PC)
        r = rank_of[ids]
        rank_of[ids] = newpos[r // 128] * 128 + (r % 128)
        tile_K[c] = tile_K[c][t_order]

    K_round = tile_K.max(axis=0)                   # [TILES, 4]

    # table row of node n
    table_row = core_of * BLKROWS + rank_of

    # --- slot lists --------------------------------------------------------
    # edges sorted by (dst core, dst tile, dst slot, window)
    dcore = core_of[dst]
    drank = rank_of[dst]
    dwin = win_of[src]
    order = np.lexsort((dwin, drank, dcore))
    src_o, dst_o = src[order], dst[order]
    dcore_o, drank_o, dwin_o = dcore[order], drank[order], dwin[order]
    loc_o = table_row[src_o] - dwin_o * WIN        # window-local row idx
    assert loc_o.min() >= 0 and loc_o.max() < WIN

    # per (dst, window) starting offset in the sorted edge array
    cnt = np.zeros((NSTAR, 4), np.int64)
    np.add.at(cnt, (dst_o, dwin_o), 1)
    # fill slot arrays per core
    DUMMY_LOC = PC                                  # same local idx all windows
    slots = []                                      # per core: [TILES][4] arrays [128, K]
    edge_ptr = 0
    # compute group starts via cumulative counts in the sorted order
    # edges are already grouped by (core, tile(slot via rank), window)
    for c in range(NC):
        core_slots = []
        for t in range(TILES):
            wslots = []
            for s in range(4):
                K = int(K_round[t, s])
                arr = np.full((128, K), DUMMY_LOC, np.int16) if K else \
                    np.zeros((128, 0), np.int16)
                wslots.append(arr)
            core_slots.append(wslots)
        slots.append(core_slots)
    # scatter edges into slots
    kfill = np.zeros((NSTAR, 4), np.int64)
    p_all = drank_o % 128
    t_all = drank_o // 128
    for i in range(len(src_o)):
        c = dcore_o[i]
        t = t_all[i]
        s = dwin_o[i]
        p = p_all[i]
        k = kfill[dst_o[i], s]
        slots[c][t][s][p, k] = loc_o[i]
        kfill[dst_o[i], s] = k + 1

    # --- pair-merged gather call structure (uniform across cores) --------
    # tiles are processed in pairs (rA, rB); the pair gather buffer is laid
    # out window-major: [s0: A-blocks,B-blocks | s1: ... | s3: ...].
    NPAIR = (TILES + 1) // 2
    calls = []            # (pr, s, pair_joff, cj) ; idx source resolved later
    pair_J = np.zeros(NPAIR, np.int64)
    tile_rng = [[None] * 4 for _ in range(TILES)]   # (pair_joff, K) per window
    for pr in range(NPAIR):
        rA, rB = 2 * pr, min(2 * pr + 1, TILES - 1)
        single = rB == rA
        joff = 0
        for s in range(4):
            KA = int(K_round[rA, s])
            KB = 0 if single else int(K_round[rB, s])
            tile_rng[rA][s] = (joff, KA)
            if not single:
                tile_rng[rB][s] = (joff + KA, KB)
            K = KA + KB
            o = 0
            while o < K:
                cj = min(MAXJ_CALL, K - o)
                calls.append((pr, s, joff + o, cj))
                o += cj
            joff += K
        pair_J[pr] = joff
    tile_J = pair_J       # (kept name for downstream sizing)

    # --- int16 wrapped idx arrays per core --------------------------------
    def wrap16(ix):                                 # [n] -> [128, n//16]
        a = ix.reshape(-1, 16).T
        return np.tile(a, (8, 1))

    # per-pair idx column ranges (identical across cores)
    tile_cols = []
    off = 0
    for pr in range(NPAIR):
        ncols = int(128 * pair_J[pr]) // 16
        tile_cols.append((off, ncols))
        off += ncols

    # per (pair, window): the pair's slot blocks = A-blocks then B-blocks
    def pair_blocks(c, pr, s):
        rA, rB = 2 * pr, min(2 * pr + 1, TILES - 1)
        bA = slots[c][rA][s]
        if rB == rA:
            return bA
        return np.concatenate([bA, slots[c][rB][s]], axis=1)

    idx_cores = []
    for c in range(NC):
        parts = []
        for (pr, s, joff, cj) in calls:
            base = tile_rng[2 * pr][s][0]           # pair_joff of window s
            js = joff - base
            blk = pair_blocks(c, pr, s)[:, js:js + cj]   # [128, cj]
            ix = blk.T.reshape(-1).astype(np.int16)
            parts.append(wrap16(ix).astype(np.int16))
        idx_cores.append(np.concatenate(parts, axis=1)
                         if parts else np.zeros((128, 0), np.int16))
    idx_all = np.stack(idx_cores)                    # [NC, 128, TOTC]

    # per-node permutation (global pi order): node at (c, rank) = ?
    pi_of = core_of * PC + rank_of                   # orig -> pi position
    inv_pi = np.empty(NSTAR, np.int64)
    inv_pi[pi_of] = np.arange(NSTAR)                 # pi position -> orig

    # pooling one-hot (per core, rank order) and counts
    batch_full = np.full(NSTAR, -1, np.int64)
    batch_full[:N] = batch
    goh = np.zeros((NC, PC, n_graphs), np.float32)
    for c in range(NC):
        b = batch_full[inv_pi[c * PC:(c + 1) * PC]]
        valid = b >= 0
        goh[c, np.arange(PC)[valid], b[valid]] = 1.0
    counts = np.maximum(np.bincount(batch, minlength=n_graphs), 1.0)

    return dict(
        N=N, NC=NC, PC=PC, NSTAR=NSTAR, TILES=TILES, BLKROWS=BLKROWS,
        WIN=WIN, K_round=K_round, tile_J=tile_J, calls=calls,
        tile_cols=tile_cols, idx_all=idx_all, inv_pi=inv_pi,
        goh=goh, counts=counts, DUMMY_LOC=DUMMY_LOC,
        NPAIR=NPAIR, pair_J=pair_J, tile_rng=tile_rng,
    )


def _augment_w(W, a_s, a_d, heads=4, hid=16):
    """[F, H*C] weights -> [F, 72] augmented (bf16): [W | Was | Wad]."""
    F = W.shape[0]
    Wr = W.reshape(F, heads, hid)
    was = np.einsum("fhc,hc->fh", Wr, a_s)
    wad = np.einsum("fhc,hc->fh", Wr, a_d)
    out = np.concatenate([W, was, wad], axis=1).astype(np.float32)
    return out.astype(BF16)


def _dummy_row():
    """[1, 80] bf16-typed bytes: h=0, as=DUMMY_AS (f32), ad=0 (f32)."""
    b = bytearray(160)
    asv = np.full(4, DUMMY_AS, np.float32)
    b[128:144] = asv.tobytes()
    return np.frombuffer(bytes(b), dtype=BF16).reshape(1, 80).copy()


# ------------------------------------------------------------- kernel build

def _build(meta, n_graphs, f_in, run_layers=3):
    import concourse.bass as bass
    import concourse.tile as tile
    from concourse import bacc, mybir
    from concourse.masks import make_identity

    NC, PC, TILES = meta["NC"], meta["PC"], meta["TILES"]
    BLKROWS, WIN = meta["BLKROWS"], meta["WIN"]
    TROWS = NC * BLKROWS
    K_round = meta["K_round"]
    calls, tile_cols = meta["calls"], meta["tile_cols"]
    NPAIR, pair_J, tile_rng = meta["NPAIR"], meta["pair_J"], meta["tile_rng"]
    TOTC = meta["idx_all"].shape[2]
    G = n_graphs
    f32, bf16, i16 = mybir.dt.float32, mybir.dt.bfloat16, mybir.dt.int16
    AX, ALU = mybir.AxisListType, mybir.AluOpType
    ACT = mybir.ActivationFunctionType

    nc = bacc.Bacc(None, target_bir_lowering=False, debug=False,
                   num_devices=NC, num_swdge_queues=NQ,
                   dynamic_dma_scratch_size=32768)

    # ---- I/O ----
    xT = nc.dram_tensor("xT", [f_in, NC * PC], bf16, kind="ExternalInput")
    idx_in = nc.dram_tensor("idx", [128, TOTC], i16, kind="ExternalInput")
    goh_in = nc.dram_tensor("goh", [PC, G], bf16, kind="ExternalInput")
    w1 = nc.dram_tensor("w1", [f_in, 72], bf16, kind="ExternalInput")
    w2 = nc.dram_tensor("w2", [64, 72], bf16, kind="ExternalInput")
    w3 = nc.dram_tensor("w3", [16, 72], bf16, kind="ExternalInput")
    b1r = nc.dram_tensor("b1r", [128, 64], f32, kind="ExternalInput")
    b2r = nc.dram_tensor("b2r", [128, 16], f32, kind="ExternalInput")
    b3r = nc.dram_tensor("b3r", [128, 16], f32, kind="ExternalInput")
    cntr = nc.dram_tensor("cntr", [16, G], f32, kind="ExternalInput")
    statsT = nc.dram_tensor("statsT", [16, G], f32, kind="ExternalInput")
    fw1 = nc.dram_tensor("fw1", [32, 32], f32, kind="ExternalInput")
    fb1 = nc.dram_tensor("fb1", [32, 1], f32, kind="ExternalInput")
    fw2 = nc.dram_tensor("fw2", [32, 16], f32, kind="ExternalInput")
    fb2 = nc.dram_tensor("fb2", [16, 1], f32, kind="ExternalInput")
    fw3 = nc.dram_tensor("fw3", [16, 1], f32, kind="ExternalInput")
    fb3 = nc.dram_tensor("fb3", [1, 1], f32, kind="ExternalInput")
    dumr = nc.dram_tensor("dumr", [1, 80], bf16, kind="ExternalInput")
    out_t = nc.dram_tensor("out", [1, G], f32, kind="ExternalOutput")

    LIN = [f_in, 64, 16]          # node-phase input width per layer
    LOUT = [64, 16, 16]           # edge-phase output width per layer

    with tile.TileContext(nc, num_cores=NC) as tc:
        with (
            tc.tile_pool(name="dram", bufs=1, space="DRAM") as dpool,
            tc.tile_pool(name="consts", bufs=1) as cpool,
            tc.tile_pool(name="nodein", bufs=1) as npool,
            tc.tile_pool(name="psum", bufs=2, space="PSUM") as ppool,
            tc.tile_pool(name="mlpp", bufs=1, space="PSUM") as mpool,
            tc.tile_pool(name="stage", bufs=3) as spool,
            tc.tile_pool(name="gat", bufs=3) as gpool,
            tc.tile_pool(name="idxp", bufs=6) as ipool,
            tc.tile_pool(name="edge", bufs=2) as epool,
            tc.tile_pool(name="poolacc", bufs=1, space="PSUM") as papool,
            tc.tile_pool(name="head", bufs=1) as hpool,
        ):
            tables = [dpool.tile([TROWS, 128], bf16, tag=f"tab{l}",
                                 name=f"table{l}") for l in range(3)]
            ag_in = [dpool.tile([64, PC], bf16, tag="agin0", name="agin0"),
                     dpool.tile([16, PC], bf16, tag="agin1", name="agin1")]
            ag_out = [dpool.tile([NC * 64, PC], bf16, tag="agout0",
                                 name="agout0", addr_space="Shared"),
                      dpool.tile([NC * 16, PC], bf16, tag="agout1",
                                 name="agout1", addr_space="Shared")]
            cc_in = dpool.tile([16, G], f32, tag="ccin")
            cc_out = dpool.tile([16, G], f32, tag="ccout",
                                addr_space="Shared")

            ident = cpool.tile([128, 128], bf16)
            make_identity(nc, ident[:])
            wsb = []
            for l, wt in enumerate((w1, w2, w3)):
                t = cpool.tile([LIN[l], 72], bf16, tag=f"w{l}", name=f"wsb{l}")
                nc.sync.dma_start(t[:], wt[:, :])
                wsb.append(t)
            brep = []
            for l, bt in enumerate((b1r, b2r, b3r)):
                t = cpool.tile([128, LOUT[l]], f32, tag=f"b{l}", name=f"bsb{l}")
                nc.sync.dma_start(t[:], bt[:, :])
                brep.append(t)
            dum_sb = cpool.tile([1, 80], bf16)
            nc.sync.dma_start(dum_sb[:], dumr[:, :])

            pid = nc.sync.partition_id()
            pool_ps = papool.tile([16, G], f32)

            qctr = [0]

            def gather_queue():
                q = qctr[0] % NQ
                qctr[0] += 1
                return q

            def leaky(dst_ap, src_ap, tmp):
                nc.vector.tensor_scalar_mul(tmp, src_ap, NEG_SLOPE)
                nc.vector.tensor_tensor(out=dst_ap, in0=src_ap, in1=tmp,
                                        op=ALU.max)

            def elu_inplace(x_ap, w, tmps):
                """x <- elu(x); x_ap f32 [128, w]; tmps pool."""
                t1 = tmps.tile([128, w], f32, tag="el1")
                t2 = tmps.tile([128, w], f32, tag="el2")
                nc.vector.tensor_scalar_min(t1[:], x_ap, 0.0)
                nc.scalar.activation(t1[:], t1[:], ACT.Exp)
                nc.vector.tensor_scalar(out=t1[:], in0=t1[:], scalar1=-1.0,
                                        scalar2=0.0, op0=ALU.add, op1=ALU.min)
                nc.vector.tensor_scalar_max(t2[:], x_ap, 0.0)
                nc.vector.tensor_tensor(out=x_ap, in0=t1[:], in1=t2[:],
                                        op=ALU.add)

            for l in range(run_layers):
                table = tables[l]
                # ---------------- node phase: build table ----------------
                for c in range(NC):
                    if l == 0:
                        xin = npool.tile([f_in, PC], bf16, tag="xin")
                        nc.sync.dma_start(xin[:], xT[:, c * PC:(c + 1) * PC])
                    else:
                        w_in = 64 if l == 1 else 16
                        xin = npool.tile([w_in, PC], bf16, tag="xin")
                        nc.sync.dma_start(
                            xin[:], ag_out[l - 1][c * w_in:(c + 1) * w_in, :])
                    for r in range(TILES):
                        ps = ppool.tile([128, 72], f32, tag="nps")
                        nc.tensor.matmul(ps[:], xin[:, r * 128:(r + 1) * 128],
                                         wsb[l][:], start=True, stop=True)
                        st = spool.tile([128, 80], bf16, tag="nst")
                        nc.scalar.copy(st[:, 0:64], ps[:, 0:64])
                        stf = st[:].bitcast(f32)
                        nc.vector.tensor_copy(stf[:, 32:40], ps[:, 64:72])
                        row0 = c * BLKROWS + r * 128
                        nc.scalar.dma_start(table[row0:row0 + 128, 0:80],
                                            st[:])
                    nc.sync.dma_start(
                        table[c * BLKROWS + PC:c * BLKROWS + PC + 1, 0:80],
                        dum_sb[:])

                # ---------------- edge phase ----------------
                W = LOUT[l]
                for pr in range(NPAIR):
                    rA = 2 * pr
                    rB = min(2 * pr + 1, TILES - 1)
                    tiles_here = [rA] if rB == rA else [rA, rB]
                    J = int(pair_J[pr])
                    coff, ncols = tile_cols[pr]
                    gat = None
                    if J > 0:
                        it = ipool.tile([128, max(ncols, 1)], i16, tag="idx")
                        nc.sync.dma_start(it[:, 0:ncols],
                                          idx_in[:, coff:coff + ncols])
                        gat = gpool.tile([128, J * 128], bf16, tag="gat")
                        g3 = gat[:].rearrange("p (j e) -> p j e", e=128)
                        ccol = 0
                        for (pr2, s_, joff, cj) in calls:
                            if pr2 != pr:
                                continue
                            n_i = 128 * cj
                            nc.gpsimd.dma_gather(
                                g3[:, joff:joff + cj, :],
                                table[s_ * WIN:(s_ + 1) * WIN, :],
                                it[:, ccol:ccol + n_i // 16],
                                n_i, n_i, 128,
                                queue_num=gather_queue(),
                                single_packet=False)
                            ccol += n_i // 16

                    for r in tiles_here:
                        rngs = [tile_rng[r][s_] for s_ in range(4)]
                        rngs = [(o, k) for (o, k) in rngs if k > 0]
                        Jt = sum(k for _, k in rngs)
                        own = epool.tile([128, 80], bf16, tag="own")
                        base = pid * BLKROWS + r * 128
                        nc.sync.dma_start(
                            own[:], table[bass.DynSlice(base, 128), 0:80])
                        ownf = own[:].bitcast(f32)
                        as_own = ownf[:, 32:36]
                        ad_own = ownf[:, 36:40]

                        if Jt > 0:
                            g3 = gat[:].rearrange("p (j e) -> p j e", e=128)
                            gf = gat[:].bitcast(f32).rearrange(
                                "p (j q) -> p j q", q=64)
                            e_t = epool.tile([128, Jt * 4], f32, tag="e")
                            e3 = e_t[:].rearrange("p (j q) -> p j q", q=4)
                            v_t = gpool.tile([128, Jt * 64], bf16, tag="vt")
                            v3 = v_t[:].rearrange("p (j h q) -> p j h q",
                                                  h=4, q=16)
                            eoff = 0
                            for (o, k) in rngs:
                                nc.vector.tensor_tensor(
                                    out=e3[:, eoff:eoff + k, :],
                                    in0=gf[:, o:o + k, 32:36],
                                    in1=ad_own.unsqueeze(1).to_broadcast(
                                        [128, k, 4]),
                                    op=ALU.add)
                                eoff += k
                            tmp_t = epool.tile([128, Jt * 4], f32, tag="etmp")
                            leaky(e_t[:], e_t[:], tmp_t[:])
                            nc.scalar.activation(e_t[:], e_t[:], ACT.Exp)
                            denom = epool.tile([128, 4], f32, tag="den")
                            nc.vector.tensor_reduce(
                                denom[:],
                                e_t[:].rearrange("p (j q) -> p q j", q=4),
                                AX.X, ALU.add)
                            eoff = 0
                            for (o, k) in rngs:
                                nc.vector.tensor_tensor(
                                    out=v3[:, eoff:eoff + k, :, :],
                                    in0=g3[:, o:o + k, 0:64].rearrange(
                                        "p j (h q) -> p j h q", q=16),
                                    in1=e3[:, eoff:eoff + k, :].unsqueeze(
                                        3).to_broadcast([128, k, 4, 16]),
                                    op=ALU.mult)
                                eoff += k

                        # self contribution
                        es = epool.tile([128, 4], f32, tag="es")
                        nc.vector.tensor_tensor(out=es[:], in0=as_own,
                                                in1=ad_own, op=ALU.add)
                        tmp4 = epool.tile([128, 4], f32, tag="es2")
                        leaky(es[:], es[:], tmp4[:])
                        nc.scalar.activation(es[:], es[:], ACT.Exp)
                        sv = epool.tile([128, 64], f32, tag="sv")
                        nc.vector.tensor_tensor(
                            out=sv[:].rearrange("p (h q) -> p h q", q=16),
                            in0=own[:, 0:64].rearrange("p (h q) -> p h q",
                                                       q=16),
                            in1=es[:].unsqueeze(2).to_broadcast([128, 4, 16]),
                            op=ALU.mult)

                        U = epool.tile([128, 64], f32, tag="U")
                        if Jt > 0:
                            cur, n, lvl = v_t, Jt, 0
                            while n > 1:
                                half, odd = n // 2, n % 2
                                nb = half + odd
                                dt_ = bf16 if lvl < 2 else f32
                                nxt = gpool.tile([128, nb * 64], dt_,
                                                 tag=f"tr{lvl % 2}_{dt_}")
                                nc.vector.tensor_tensor(
                                    out=nxt[:, 0:half * 64],
                                    in0=cur[:, 0:half * 64],
                                    in1=cur[:, half * 64:2 * half * 64],
                                    op=ALU.add)
                                if odd:
                                    nc.vector.tensor_copy(
                                        nxt[:, half * 64:nb * 64],
                                        cur[:, 2 * half * 64:n * 64])
                                cur, n, lvl = nxt, nb, lvl + 1
                            nc.vector.tensor_tensor(out=U[:], in0=cur[:, 0:64],
                                                    in1=sv[:], op=ALU.add)
                            dfull = epool.tile([128, 4], f32, tag="dful")
                            nc.vector.tensor_tensor(out=dfull[:], in0=denom[:],
                                                    in1=es[:], op=ALU.add)
                        else:
                            nc.vector.tensor_copy(U[:], sv[:])
                            dfull = es

                        recip = epool.tile([128, 4], f32, tag="rec")
                        nc.vector.reciprocal(recip[:], dfull[:])
                        if l > 0:
                            nc.vector.tensor_scalar_mul(recip[:], recip[:],
                                                        0.25)
                        o64 = epool.tile([128, 64], f32, tag="o64")
                        nc.vector.tensor_tensor(
                            out=o64[:].rearrange("p (h q) -> p h q", q=16),
                            in0=U[:].rearrange("p (h q) -> p h q", q=16),
                            in1=recip[:].unsqueeze(2).to_broadcast(
                                [128, 4, 16]),
                            op=ALU.mult)
                        if l == 0:
                            nc.vector.tensor_tensor(out=o64[:], in0=o64[:],
                                                    in1=brep[0][:], op=ALU.add)
                            elu_inplace(o64[:], 64, epool)
                            xnext = epool.tile([128, 64], bf16, tag="xn")
                            nc.vector.tensor_copy(xnext[:], o64[:])
                        else:
                            o16 = epool.tile([128, 16], f32, tag="o16")
                            nc.vector.tensor_reduce(
                                o16[:],
                                o64[:].rearrange("p (h q) -> p q h", q=16),
                                AX.X, ALU.add)
                            nc.vector.tensor_tensor(out=o16[:], in0=o16[:],
                                                    in1=brep[l][:], op=ALU.add)
                            if l == 1:
                                elu_inplace(o16[:], 16, epool)
                            xnext = epool.tile([128, 16], bf16, tag="xn16")
                            nc.vector.tensor_copy(xnext[:], o16[:])

                        if l < 2:
                            wout = 64 if l == 0 else 16
                            pst = ppool.tile([wout, 128], bf16, tag="pst")
                            nc.tensor.transpose(out=pst[:], in_=xnext[:],
                                                identity=ident[:])
                            stt = spool.tile([wout, 128], bf16, tag="stt")
                            nc.scalar.copy(stt[:], pst[:])
                            nc.sync.dma_start(
                                ag_in[l][:, r * 128:(r + 1) * 128], stt[:])
                        else:
                            gt = epool.tile([128, G], bf16, tag="goh")
                            nc.sync.dma_start(
                                gt[:], goh_in[r * 128:(r + 1) * 128, :])
                            nc.tensor.matmul(pool_ps[:], xnext[:], gt[:],
                                             start=(r == 0),
                                             stop=(r == TILES - 1))

                if l < 2 and run_layers > l + 1:
                    nc.gpsimd.collective_compute(
                        "AllGather", mybir.AluOpType.bypass,
                        replica_groups=[list(range(NC))],
                        ins=[ag_in[l].opt()], outs=[ag_out[l].opt()])

            # ---------------- pooling + MLP head ----------------
            if run_layers == 3:
                pooled = hpool.tile([16, G], f32, tag="pooled")
                nc.scalar.copy(pooled[:], pool_ps[:])
                nc.sync.dma_start(cc_in[:, :], pooled[:])
                nc.gpsimd.collective_compute(
                    "AllReduce", mybir.AluOpType.add,
                    replica_groups=[list(range(NC))],
                    ins=[cc_in.opt()], outs=[cc_out.opt()])
                zt = hpool.tile([32, G], f32, tag="zt")
                nc.sync.dma_start(zt[0:16, :], cc_out[:, :])
                cr = hpool.tile([16, G], f32, tag="cr")
                nc.sync.dma_start(cr[:], cntr[:, :])
                nc.vector.tensor_tensor(out=zt[0:16, :], in0=zt[0:16, :],
                                        in1=cr[:], op=ALU.mult)
                nc.sync.dma_start(zt[16:32, :], statsT[:, :])
                fw1s = hpool.tile([32, 32], f32, tag="fw1")
                nc.sync.dma_start(fw1s[:], fw1[:, :])
                fb1s = hpool.tile([32, 1], f32, tag="fb1")
                nc.sync.dma_start(fb1s[:], fb1[:, :])
                fw2s = hpool.tile([32, 16], f32, tag="fw2")
                nc.sync.dma_start(fw2s[:], fw2[:, :])
                fb2s = hpool.tile([16, 1], f32, tag="fb2")
                nc.sync.dma_start(fb2s[:], fb2[:, :])
                fw3s = hpool.tile([16, 1], f32, tag="fw3")
                nc.sync.dma_start(fw3s[:], fw3[:, :])
                fb3s = hpool.tile([1, 1], f32, tag="fb3")
                nc.sync.dma_start(fb3s[:], fb3[:, :])

                mp1 = mpool.tile([32, G], f32, tag="mp1")
                nc.tensor.matmul(mp1[:], fw1s[:], zt[:], start=True, stop=True)
                h1 = hpool.tile([32, G], f32, tag="h1")
                nc.scalar.activation(h1[:], mp1[:], ACT.Relu, bias=fb1s[:, 0:1])
                mp2 = mpool.tile([16, G], f32, tag="mp2")
                nc.tensor.matmul(mp2[:], fw2s[:], h1[:], start=True, stop=True)
                h2 = hpool.tile([16, G], f32, tag="h2")
                nc.scalar.activation(h2[:], mp2[:], ACT.Relu, bias=fb2s[:, 0:1])
                mp3 = mpool.tile([1, G], f32, tag="mp3")
                nc.tensor.matmul(mp3[:], fw3s[:], h2[:], start=True, stop=True)
                ot = hpool.tile([1, G], f32, tag="ot")
                nc.vector.tensor_tensor(
                    out=ot[:], in0=mp3[:],
                    in1=fb3s[:, 0:1].to_broadcast([1, G]), op=ALU.add)
                nc.sync.dma_start(out_t[:, :], ot[:])

    nc.finalize()
    return nc


# ------------------------------------------------------------------- driver

def run_gat(x, stats, W1, a1s, a1d, b1, W2, a2s, a2d, b2, W3, a3s, a3d, b3,
            fw1, fb1, fw2, fb2, fw3, fb3, edge_index, batch,
            trace=False, _cache={}):
    from concourse.bass_utils import run_bass_kernel_spmd

    x = np.asarray(x, np.float32)
    stats = np.asarray(stats, np.float32)
    n_graphs = stats.shape[0]
    f_in = x.shape[1]
    meta = _prep(x, np.asarray(edge_index), np.asarray(batch), n_graphs)
    NC, PC, NSTAR = meta["NC"], meta["PC"], meta["NSTAR"]

    nc = _build(meta, n_graphs, f_in)

    # host-side input prep
    inv_pi = meta["inv_pi"]
    xs = np.zeros((NSTAR, f_in), np.float32)
    xs[:x.shape[0]] = x
    xT = np.ascontiguousarray(xs[inv_pi].T).astype(BF16)

    cntrep = np.tile((1.0 / meta["counts"]).astype(np.float32)[None, :],
                     (16, 1))
    in_common = dict(
        xT=xT,
        w1=_augment_w(np.asarray(W1, np.float32), np.asarray(a1s, np.float32),
                      np.asarray(a1d, np.float32)),
        w2=_augment_w(np.asarray(W2, np.float32), np.asarray(a2s, np.float32),
                      np.asarray(a2d, np.float32)),
        w3=_augment_w(np.asarray(W3, np.float32), np.asarray(a3s, np.float32),
                      np.asarray(a3d, np.float32)),
        b1r=np.tile(np.asarray(b1, np.float32)[None, :], (128, 1)),
        b2r=np.tile(np.asarray(b2, np.float32)[None, :], (128, 1)),
        b3r=np.tile(np.asarray(b3, np.float32)[None, :], (128, 1)),
        cntr=cntrep.astype(np.float32),
        statsT=np.ascontiguousarray(stats.T).astype(np.float32),
        fw1=np.asarray(fw1, np.float32),
        fb1=np.asarray(fb1, np.float32).reshape(32, 1),
        fw2=np.asarray(fw2, np.float32),
        fb2=np.asarray(fb2, np.float32).reshape(16, 1),
        fw3=np.asarray(fw3, np.float32),
        fb3=np.asarray(fb3, np.float32).reshape(1, 1),
        dumr=_dummy_row(),
    )
    in_maps = []
    for c in range(NC):
        m = dict(in_common)
        m["idx"] = np.ascontiguousarray(meta["idx_all"][c])
        m["goh"] = meta["goh"][c].astype(BF16)
        in_maps.append(m)

    res = run_bass_kernel_spmd(nc, in_maps, list(range(NC)), trace=trace)
    out = res.results[0]["out"]                      # [1, G]
    return np.ascontiguousarray(out.T).astype(np.float32), res


def kernel(**inputs):
    out, _ = run_gat(**inputs)
    return out



# revision 6
# speedup vs baseline: 2.0988x; 2.0988x over previous
"""GAT (3-layer, PyG-style) forward on 8 Trainium2 NeuronCores.

Strategy (v2):
  - Node space padded to 8*PC nodes; core c owns nodes [c*PC, (c+1)*PC).
  - Per layer, a "table" in DRAM holds one 256B row per node:
      [h (64 bf16) | as (4 f32) | ad (4 f32) | pad].
  - Node phase: each core computes ONLY ITS OWN block's rows (layer 0 from
    its x slice; layers 1-2 fused into the edge-phase epilogue), then an
    AllGather collective replicates the full table to every core.
  - Edge phase (per core, per 128-node tile): dma_gather the rows of all
    in-neighbours (4 int16-addressable windows of the table), compute
    e = as_src + ad_dst on DVE (head-major compaction), leakyrelu + exp on
    the Scalar engine (Lrelu/Exp, one act table), denom via contiguous
    reduce, U = sum ex*h via an all-bf16 halving tree; self-loop handled
    densely from the node's own row.
  - out = U/denom (mean over heads for concat=False) + bias (+elu); for
    l<2 the epilogue immediately computes the NEXT layer's augmented
    node-phase matmul (transpose + matmul) and writes next-layer table rows.
  - Final: per-tile pooling matmul into PSUM, AllReduce, MLP head on-device.

Host does: graph preprocessing (degree profiles, tile clustering by
max-window-degree with fuller-bin preference, slot/pad assignment, int16
index arrays), weight augmentation, and the final [1,64] -> [64,1] reshape.
"""

import sys

sys.path.insert(0, "/opt/trn_rl_repo")

import numpy as np
import ml_dtypes

BF16 = ml_dtypes.bfloat16

NEG_SLOPE = 0.2
DUMMY_AS = -30000.0
MAXJ_CALL = 14  # <=1792 idx per dma_gather (ring 2048 w/ 32KB scratch)
NQ = 4         # SWDGE queues


# ----------------------------------------------------------------- host prep

def _prep(x, edge_index, batch, n_graphs):
    """Graph preprocessing. Returns a dict of host arrays + structure."""
    N = x.shape[0]
    NC = 8
    PC = int(np.ceil(N / NC / 128)) * 128          # nodes per core (padded)
    NSTAR = NC * PC
    TILES = PC // 128
    BLKROWS = PC + 1                               # +1 dummy row per core blk
    WIN = 2 * BLKROWS                              # gather window (2 blocks)
    assert WIN <= 32767

    src = edge_index[0].astype(np.int64)
    dst = edge_index[1].astype(np.int64)

    core_of = np.arange(NSTAR) // PC               # orig id -> core
    win_of = (core_of // 2).astype(np.int64)       # orig id -> window

    # per-dst in-degree per window (real edges only; self-loops added densely)
    degw = np.zeros((NSTAR, 4), np.int64)
    np.add.at(degw, (dst, win_of[src]), 1)

    # --- cluster nodes into tiles (per core) by window-degree profile ----
    # Greedy: place nodes in order of descending max window-degree into the
    # bin where the sum-of-window-maxima grows least; prefer fuller bins on
    # ties (concentrates padding).  ~1.56x padding vs 2.48x for sum-ordered.
    rank_of = np.empty(NSTAR, np.int64)
    tile_K = np.zeros((NC, TILES, 4), np.int64)    # per-core per-tile max deg
    for c in range(NC):
        ids = np.arange(c * PC, (c + 1) * PC)
        prof = degw[ids].astype(np.int64)
        order0 = np.argsort(-prof.max(axis=1), kind="stable")
        bins_max = np.zeros((TILES, 4), np.int64)
        bins_cnt = np.zeros(TILES, np.int64)
        assign = np.empty(PC, np.int64)
        slot_in = np.empty(PC, np.int64)
        for j in order0:
            v = prof[j]
            delta = np.maximum(bins_max, v).sum(axis=1) - bins_max.sum(axis=1)
            score = delta * 1000 - bins_cnt
            score = score + (bins_cnt >= 128) * (1 << 40)
            b_ = int(np.argmin(score))
            assign[j] = b_
            slot_in[j] = bins_cnt[b_]
            bins_cnt[b_] += 1
            bins_max[b_] = np.maximum(bins_max[b_], v)
        rank_of[ids] = assign * 128 + slot_in
        tile_K[c] = bins_max

    # order each core's tiles by K-profile (descending total) so that the
    # r-th tile of each core has a similar profile across cores (SPMD
    # uniformity: round r uses K_s(r) = max over cores).
    for c in range(NC):
        tot = tile_K[c].sum(axis=1)
        t_order = np.argsort(-tot, kind="stable")
        newpos = np.empty(TILES, np.int64)
        newpos[t_order] = np.arange(TILES)
        ids = np.arange(c * PC, (c + 1) * PC)
        r = rank_of[ids]
        rank_of[ids] = newpos[r // 128] * 128 + (r % 128)
        tile_K[c] = tile_K[c][t_order]

    K_round = tile_K.max(axis=0)                   # [TILES, 4]

    # table row of node n
    table_row = core_of * BLKROWS + rank_of

    # --- slot lists --------------------------------------------------------
    dcore = core_of[dst]
    drank = rank_of[dst]
    dwin = win_of[src]
    order = np.lexsort((dwin, drank, dcore))
    src_o, dst_o = src[order], dst[order]
    dcore_o, drank_o, dwin_o = dcore[order], drank[order], dwin[order]
    loc_o = table_row[src_o] - dwin_o * WIN        # window-local row idx
    assert loc_o.min() >= 0 and loc_o.max() < WIN

    DUMMY_LOC = PC                                  # same local idx all windows
    slots = []                                      # per core: [TILES][4] arrays [128, K]
    for c in range(NC):
        core_slots = []
        for t in range(TILES):
            wslots = []
            for s in range(4):
                K = int(K_round[t, s])
                arr = np.full((128, K), DUMMY_LOC, np.int16) if K else \
                    np.zeros((128, 0), np.int16)
                wslots.append(arr)
            core_slots.append(wslots)
        slots.append(core_slots)
    # scatter edges into slots
    kfill = np.zeros((NSTAR, 4), np.int64)
    p_all = drank_o % 128
    t_all = drank_o // 128
    for i in range(len(src_o)):
        c = dcore_o[i]
        t = t_all[i]
        s = dwin_o[i]
        p = p_all[i]
        k = kfill[dst_o[i], s]
        slots[c][t][s][p, k] = loc_o[i]
        kfill[dst_o[i], s] = k + 1

    # --- pair-merged gather call structure (uniform across cores) --------
    NPAIR = (TILES + 1) // 2
    calls = []            # (pr, s, pair_joff, cj)
    pair_J = np.zeros(NPAIR, np.int64)
    tile_rng = [[None] * 4 for _ in range(TILES)]   # (pair_joff, K) per window
    for pr in range(NPAIR):
        rA, rB = 2 * pr, min(2 * pr + 1, TILES - 1)
        single = rB == rA
        joff = 0
        for s in range(4):
            KA = int(K_round[rA, s])
            KB = 0 if single else int(K_round[rB, s])
            tile_rng[rA][s] = (joff, KA)
            if not single:
                tile_rng[rB][s] = (joff + KA, KB)
            K = KA + KB
            o = 0
            while o < K:
                cj = min(MAXJ_CALL, K - o)
                calls.append((pr, s, joff + o, cj))
                o += cj
            joff += K
        pair_J[pr] = joff
    tile_J = pair_J

    # --- int16 wrapped idx arrays per core --------------------------------
    def wrap16(ix):                                 # [n] -> [128, n//16]
        a = ix.reshape(-1, 16).T
        return np.tile(a, (8, 1))

    tile_cols = []
    off = 0
    for pr in range(NPAIR):
        ncols = int(128 * pair_J[pr]) // 16
        tile_cols.append((off, ncols))
        off += ncols

    def pair_blocks(c, pr, s):
        rA, rB = 2 * pr, min(2 * pr + 1, TILES - 1)
        bA = slots[c][rA][s]
        if rB == rA:
            return bA
        return np.concatenate([bA, slots[c][rB][s]], axis=1)

    idx_cores = []
    for c in range(NC):
        parts = []
        for (pr, s, joff, cj) in calls:
            base = tile_rng[2 * pr][s][0]
            js = joff - base
            blk = pair_blocks(c, pr, s)[:, js:js + cj]   # [128, cj]
            ix = blk.T.reshape(-1).astype(np.int16)
            parts.append(wrap16(ix).astype(np.int16))
        idx_cores.append(np.concatenate(parts, axis=1)
                         if parts else np.zeros((128, 0), np.int16))
    idx_all = np.stack(idx_cores)                    # [NC, 128, TOTC]

    # per-node permutation (global pi order)
    pi_of = core_of * PC + rank_of                   # orig -> pi position
    inv_pi = np.empty(NSTAR, np.int64)
    inv_pi[pi_of] = np.arange(NSTAR)                 # pi position -> orig

    # pooling one-hot (per core, rank order) and counts
    batch_full = np.full(NSTAR, -1, np.int64)
    batch_full[:N] = batch
    goh = np.zeros((NC, PC, n_graphs), np.float32)
    for c in range(NC):
        b = batch_full[inv_pi[c * PC:(c + 1) * PC]]
        valid = b >= 0
        goh[c, np.arange(PC)[valid], b[valid]] = 1.0
    counts = np.maximum(np.bincount(batch, minlength=n_graphs), 1.0)

    return dict(
        N=N, NC=NC, PC=PC, NSTAR=NSTAR, TILES=TILES, BLKROWS=BLKROWS,
        WIN=WIN, K_round=K_round, tile_J=tile_J, calls=calls,
        tile_cols=tile_cols, idx_all=idx_all, inv_pi=inv_pi,
        goh=goh, counts=counts, DUMMY_LOC=DUMMY_LOC,
        NPAIR=NPAIR, pair_J=pair_J, tile_rng=tile_rng,
    )


def _augment_w(W, a_s, a_d, heads=4, hid=16):
    """[F, H*C] weights -> [F, 72] augmented (bf16): [W | Was | Wad]."""
    F = W.shape[0]
    Wr = W.reshape(F, heads, hid)
    was = np.einsum("fhc,hc->fh", Wr, a_s)
    wad = np.einsum("fhc,hc->fh", Wr, a_d)
    out = np.concatenate([W, was, wad], axis=1).astype(np.float32)
    return out.astype(BF16)


def _dummy_row():
    """[1, 80] bf16-typed bytes: h=0, as=DUMMY_AS (f32), ad=0 (f32)."""
    b = bytearray(160)
    asv = np.full(4, DUMMY_AS, np.float32)
    b[128:144] = asv.tobytes()
    return np.frombuffer(bytes(b), dtype=BF16).reshape(1, 80).copy()


# ------------------------------------------------------------- kernel build

def _build(meta, n_graphs, f_in, run_layers=3):
    import concourse.bass as bass
    import concourse.tile as tile
    from concourse import bacc, mybir
    from concourse.masks import make_identity

    NC, PC, TILES = meta["NC"], meta["PC"], meta["TILES"]
    BLKROWS, WIN = meta["BLKROWS"], meta["WIN"]
    TROWS = NC * BLKROWS
    K_round = meta["K_round"]
    calls, tile_cols = meta["calls"], meta["tile_cols"]
    NPAIR, pair_J, tile_rng = meta["NPAIR"], meta["pair_J"], meta["tile_rng"]
    TOTC = meta["idx_all"].shape[2]
    G = n_graphs
    f32, bf16, i16 = mybir.dt.float32, mybir.dt.bfloat16, mybir.dt.int16
    AX, ALU = mybir.AxisListType, mybir.AluOpType
    ACT = mybir.ActivationFunctionType

    nc = bacc.Bacc(None, target_bir_lowering=False, debug=False,
                   num_devices=NC, num_swdge_queues=NQ,
                   dynamic_dma_scratch_size=32768)

    # ---- I/O ----
    xTl = nc.dram_tensor("xTl", [f_in, PC], bf16, kind="ExternalInput")
    idx_in = nc.dram_tensor("idx", [128, TOTC], i16, kind="ExternalInput")
    goh_in = nc.dram_tensor("goh", [PC, G], bf16, kind="ExternalInput")
    w1 = nc.dram_tensor("w1", [f_in, 72], bf16, kind="ExternalInput")
    w2 = nc.dram_tensor("w2", [64, 72], bf16, kind="ExternalInput")
    w3 = nc.dram_tensor("w3", [16, 72], bf16, kind="ExternalInput")
    b1r = nc.dram_tensor("b1r", [128, 64], f32, kind="ExternalInput")
    b2r = nc.dram_tensor("b2r", [128, 16], f32, kind="ExternalInput")
    b3r = nc.dram_tensor("b3r", [128, 16], f32, kind="ExternalInput")
    cntr = nc.dram_tensor("cntr", [16, G], f32, kind="ExternalInput")
    statsT = nc.dram_tensor("statsT", [16, G], f32, kind="ExternalInput")
    fw1 = nc.dram_tensor("fw1", [32, 32], f32, kind="ExternalInput")
    fb1 = nc.dram_tensor("fb1", [32, 1], f32, kind="ExternalInput")
    fw2 = nc.dram_tensor("fw2", [32, 16], f32, kind="ExternalInput")
    fb2 = nc.dram_tensor("fb2", [16, 1], f32, kind="ExternalInput")
    fw3 = nc.dram_tensor("fw3", [16, 1], f32, kind="ExternalInput")
    fb3 = nc.dram_tensor("fb3", [1, 1], f32, kind="ExternalInput")
    dumr = nc.dram_tensor("dumr", [1, 80], bf16, kind="ExternalInput")
    out_t = nc.dram_tensor("out", [1, G], f32, kind="ExternalOutput")

    LIN = [f_in, 64, 16]          # node-phase input width per layer
    LOUT = [64, 16, 16]           # edge-phase output width per layer

    with tile.TileContext(nc, num_cores=NC) as tc:
        with (
            tc.tile_pool(name="dram", bufs=1, space="DRAM") as dpool,
            tc.tile_pool(name="consts", bufs=1) as cpool,
            tc.tile_pool(name="nodein", bufs=1) as npool,
            tc.tile_pool(name="psum", bufs=2, space="PSUM") as ppool,
            tc.tile_pool(name="mlpp", bufs=1, space="PSUM") as mpool,
            tc.tile_pool(name="stage", bufs=3) as spool,
            tc.tile_pool(name="gat", bufs=3) as gpool,
            tc.tile_pool(name="idxp", bufs=6) as ipool,
            tc.tile_pool(name="edge", bufs=2) as epool,
            tc.tile_pool(name="poolacc", bufs=1, space="PSUM") as papool,
            tc.tile_pool(name="head", bufs=1) as hpool,
        ):
            # own-block table inputs (local) + AllGather'd full tables
            blk_in = [dpool.tile([BLKROWS, 128], bf16, tag=f"blk{l}",
                                 name=f"blkin{l}") for l in range(3)]
            tables = [dpool.tile([TROWS, 128], bf16, tag=f"tab{l}",
                                 name=f"table{l}", addr_space="Shared")
                      for l in range(3)]
            cc_in = dpool.tile([16, G], f32, tag="ccin")
            cc_out = dpool.tile([16, G], f32, tag="ccout",
                                addr_space="Shared")

            ident = cpool.tile([128, 128], bf16)
            make_identity(nc, ident[:])
            wsb = []
            for l, wt in enumerate((w1, w2, w3)):
                t = cpool.tile([LIN[l], 72], bf16, tag=f"w{l}", name=f"wsb{l}")
                nc.sync.dma_start(t[:], wt[:, :])
                wsb.append(t)
            brep = []
            for l, bt in enumerate((b1r, b2r, b3r)):
                t = cpool.tile([128, LOUT[l]], f32, tag=f"b{l}", name=f"bsb{l}")
                nc.sync.dma_start(t[:], bt[:, :])
                brep.append(t)
            dum_sb = cpool.tile([1, 80], bf16)
            nc.sync.dma_start(dum_sb[:], dumr[:, :])

            pid = nc.sync.partition_id()
            pool_ps = papool.tile([16, G], f32)

            qctr = [0]

            def gather_queue():
                q = qctr[0] % NQ
                qctr[0] += 1
                return q

            def elu_inplace(x_ap, w, tmps):
                """x <- elu(x); x_ap f32 [128, w]; tmps pool."""
                t1 = tmps.tile([128, w], f32, tag="el1")
                t2 = tmps.tile([128, w], f32, tag="el2")
                nc.vector.tensor_scalar_min(t1[:], x_ap, 0.0)
                nc.scalar.activation(t1[:], t1[:], ACT.Exp)
                nc.vector.tensor_scalar(out=t1[:], in0=t1[:], scalar1=-1.0,
                                        scalar2=0.0, op0=ALU.add, op1=ALU.min)
                nc.vector.tensor_scalar_max(t2[:], x_ap, 0.0)
                nc.vector.tensor_tensor(out=x_ap, in0=t1[:], in1=t2[:],
                                        op=ALU.add)

            def pack_row(st, ps):
                """Pack PSUM [128,72] f32 -> SBUF [128,80] bf16 table row."""
                nc.scalar.copy(st[:, 0:64], ps[:, 0:64])
                stf = st[:].bitcast(f32)
                nc.vector.tensor_copy(stf[:, 32:40], ps[:, 64:72])

            # ---------------- node phase: layer 0, own block only ----------
            xin = npool.tile([f_in, PC], bf16, tag="xin")
            nc.sync.dma_start(xin[:], xTl[:, :])
            for r in range(TILES):
                ps = ppool.tile([128, 72], f32, tag="nps")
                nc.tensor.matmul(ps[:], xin[:, r * 128:(r + 1) * 128],
                                 wsb[0][:], start=True, stop=True)
                st = spool.tile([128, 80], bf16, tag="nst")
                pack_row(st, ps[:])
                nc.scalar.dma_start(blk_in[0][r * 128:r * 128 + 128, 0:80],
                                    st[:])
            nc.sync.dma_start(blk_in[0][PC:PC + 1, 0:80], dum_sb[:])
            nc.gpsimd.collective_compute(
                "AllGather", mybir.AluOpType.bypass,
                replica_groups=[list(range(NC))],
                ins=[blk_in[0].opt()], outs=[tables[0].opt()])

            for l in range(run_layers):
                table = tables[l]
                W = LOUT[l]
                for pr in range(NPAIR):
                    rA = 2 * pr
                    rB = min(2 * pr + 1, TILES - 1)
                    tiles_here = [rA] if rB == rA else [rA, rB]
                    J = int(pair_J[pr])
                    coff, ncols = tile_cols[pr]
                    gat = None
                    if J > 0:
                        it = ipool.tile([128, max(ncols, 1)], i16, tag="idx")
                        nc.sync.dma_start(it[:, 0:ncols],
                                          idx_in[:, coff:coff + ncols])
                        gat = gpool.tile([128, J * 128], bf16, tag="gat")
                        g3 = gat[:].rearrange("p (j e) -> p j e", e=128)
                        ccol = 0
                        for (pr2, s_, joff, cj) in calls:
                            if pr2 != pr:
                                continue
                            n_i = 128 * cj
                            nc.gpsimd.dma_gather(
                                g3[:, joff:joff + cj, :],
                                table[s_ * WIN:(s_ + 1) * WIN, :],
                                it[:, ccol:ccol + n_i // 16],
                                n_i, n_i, 128,
                                queue_num=gather_queue(),
                                single_packet=False)
                            ccol += n_i // 16

                    for r in tiles_here:
                        rngs = [tile_rng[r][s_] for s_ in range(4)]
                        rngs = [(o, k) for (o, k) in rngs if k > 0]
                        Jt = sum(k for _, k in rngs)
                        own = epool.tile([128, 80], bf16, tag="own")
                        base = pid * BLKROWS + r * 128
                        nc.sync.dma_start(
                            own[:], table[bass.DynSlice(base, 128), 0:80])
                        ownf = own[:].bitcast(f32)
                        as_own = ownf[:, 32:36]
                        ad_own = ownf[:, 36:40]

                        if Jt > 0:
                            g3 = gat[:].rearrange("p (j e) -> p j e", e=128)
                            gf = gat[:].bitcast(f32).rearrange(
                                "p (j q) -> p j q", q=64)
                            # e-stage, head-major: e[p, h, j] = as_src + ad_dst
                            e_t = epool.tile([128, 4 * Jt], f32, tag="e")
                            e3 = e_t[:].rearrange("p (q j) -> p q j", q=4)
                            o2 = 0
                            for (o, k) in rngs:
                                nc.vector.tensor_tensor(
                                    out=e3[:, :, o2:o2 + k],
                                    in0=gf[:, o:o + k, 32:36].rearrange(
                                        "p k q -> p q k"),
                                    in1=ad_own.unsqueeze(2).to_broadcast(
                                        [128, 4, k]),
                                    op=ALU.add)
                                o2 += k
                            nc.scalar.activation(e_t[:], e_t[:], ACT.Lrelu,
                                                 alpha=NEG_SLOPE)
                            ex_t = epool.tile([128, 4 * Jt], bf16, tag="ex")
                            nc.scalar.activation(ex_t[:], e_t[:], ACT.Exp)
                            ex3 = ex_t[:].rearrange("p (q j) -> p q j", q=4)
                            denom = epool.tile([128, 4], f32, tag="den")
                            nc.vector.tensor_reduce(
                                denom[:], ex3, AX.X, ALU.add)
                            # v[p, j, h, q16] = h_src * ex
                            v_t = gpool.tile([128, Jt * 64], bf16, tag="vt")
                            v3 = v_t[:].rearrange("p (j h q) -> p j h q",
                                                  h=4, q=16)
                            o2 = 0
                            for (o, k) in rngs:
                                nc.vector.tensor_tensor(
                                    out=v3[:, o2:o2 + k, :, :],
                                    in0=g3[:, o:o + k, 0:64].rearrange(
                                        "p j (h q) -> p j h q", q=16),
                                    in1=ex3[:, :, o2:o2 + k].rearrange(
                                        "p q k -> p k q").unsqueeze(
                                        3).to_broadcast([128, k, 4, 16]),
                                    op=ALU.mult)
                                o2 += k

                        # self contribution
                        es = epool.tile([128, 4], f32, tag="es")
                        nc.vector.tensor_tensor(out=es[:], in0=as_own,
                                                in1=ad_own, op=ALU.add)
                        nc.scalar.activation(es[:], es[:], ACT.Lrelu,
                                             alpha=NEG_SLOPE)
                        nc.scalar.activation(es[:], es[:], ACT.Exp)
                        sv = epool.tile([128, 64], f32, tag="sv")
                        nc.vector.tensor_tensor(
                            out=sv[:].rearrange("p (h q) -> p h q", q=16),
                            in0=own[:, 0:64].rearrange("p (h q) -> p h q",
                                                       q=16),
                            in1=es[:].unsqueeze(2).to_broadcast([128, 4, 16]),
                            op=ALU.mult)

                        U = epool.tile([128, 64], f32, tag="U")
                        if Jt > 0:
                            cur, n, lvl = v_t, Jt, 0
                            while n > 1:
                                half, odd = n // 2, n % 2
                                nb = half + odd
                                nxt = gpool.tile([128, nb * 64], bf16,
                                                 tag=f"tr{lvl % 2}")
                                nc.vector.tensor_tensor(
                                    out=nxt[:, 0:half * 64],
                                    in0=cur[:, 0:half * 64],
                                    in1=cur[:, half * 64:2 * half * 64],
                                    op=ALU.add)
                                if odd:
                                    nc.vector.tensor_copy(
                                        nxt[:, half * 64:nb * 64],
                                        cur[:, 2 * half * 64:n * 64])
                                cur, n, lvl = nxt, nb, lvl + 1
                            nc.vector.tensor_tensor(out=U[:], in0=cur[:, 0:64],
                                                    in1=sv[:], op=ALU.add)
                            dfull = epool.tile([128, 4], f32, tag="dful")
                            nc.vector.tensor_tensor(out=dfull[:], in0=denom[:],
                                                    in1=es[:], op=ALU.add)
                        else:
                            nc.vector.tensor_copy(U[:], sv[:])
                            dfull = es

                        recip = epool.tile([128, 4], f32, tag="rec")
                        nc.vector.reciprocal(recip[:], dfull[:])
                        if l > 0:
                            nc.vector.tensor_scalar_mul(recip[:], recip[:],
                                                        0.25)
                        o64 = epool.tile([128, 64], f32, tag="o64")
                        nc.vector.tensor_tensor(
                            out=o64[:].rearrange("p (h q) -> p h q", q=16),
                            in0=U[:].rearrange("p (h q) -> p h q", q=16),
                            in1=recip[:].unsqueeze(2).to_broadcast(
                                [128, 4, 16]),
                            op=ALU.mult)
                        if l == 0:
                            nc.vector.tensor_tensor(out=o64[:], in0=o64[:],
                                                    in1=brep[0][:], op=ALU.add)
                            elu_inplace(o64[:], 64, epool)
                            xnext = epool.tile([128, 64], bf16, tag="xn")
                            nc.vector.tensor_copy(xnext[:], o64[:])
                        else:
                            o16 = epool.tile([128, 16], f32, tag="o16")
                            nc.vector.tensor_reduce(
                                o16[:],
                                o64[:].rearrange("p (h q) -> p q h", q=16),
                                AX.X, ALU.add)
                            nc.vector.tensor_tensor(out=o16[:], in0=o16[:],
                                                    in1=brep[l][:], op=ALU.add)
                            if l == 1:
                                elu_inplace(o16[:], 16, epool)
                            xnext = epool.tile([128, 16], bf16, tag="xn16")
                            nc.vector.tensor_copy(xnext[:], o16[:])

                        if l < 2:
                            # fused next-layer node phase for own rows
                            wout = 64 if l == 0 else 16
                            pst = ppool.tile([wout, 128], bf16, tag="pst")
                            nc.tensor.transpose(out=pst[:], in_=xnext[:],
                                                identity=ident[:])
                            stt = spool.tile([wout, 128], bf16, tag="stt")
                            nc.scalar.copy(stt[:], pst[:])
                            ps2 = ppool.tile([128, 72], f32, tag="nps")
                            nc.tensor.matmul(ps2[:], stt[:], wsb[l + 1][:],
                                             start=True, stop=True)
                            st2 = spool.tile([128, 80], bf16, tag="nst2")
                            pack_row(st2, ps2[:])
                            nc.scalar.dma_start(
                                blk_in[l + 1][r * 128:r * 128 + 128, 0:80],
                                st2[:])
                        else:
                            gt = epool.tile([128, G], bf16, tag="goh")
                            nc.sync.dma_start(
                                gt[:], goh_in[r * 128:(r + 1) * 128, :])
                            nc.tensor.matmul(pool_ps[:], xnext[:], gt[:],
                                             start=(r == 0),
                                             stop=(r == TILES - 1))

                if l < 2 and run_layers > l + 1:
                    nc.sync.dma_start(blk_in[l + 1][PC:PC + 1, 0:80],
                                      dum_sb[:])
                    nc.gpsimd.collective_compute(
                        "AllGather", mybir.AluOpType.bypass,
                        replica_groups=[list(range(NC))],
                        ins=[blk_in[l + 1].opt()], outs=[tables[l + 1].opt()])

            # ---------------- pooling + MLP head ----------------
            if run_layers == 3:
                pooled = hpool.tile([16, G], f32, tag="pooled")
                nc.scalar.copy(pooled[:], pool_ps[:])
                nc.sync.dma_start(cc_in[:, :], pooled[:])
                nc.gpsimd.collective_compute(
                    "AllReduce", mybir.AluOpType.add,
                    replica_groups=[list(range(NC))],
                    ins=[cc_in.opt()], outs=[cc_out.opt()])
                zt = hpool.tile([32, G], f32, tag="zt")
                nc.sync.dma_start(zt[0:16, :], cc_out[:, :])
                cr = hpool.tile([16, G], f32, tag="cr")
                nc.sync.dma_start(cr[:], cntr[:, :])
                nc.vector.tensor_tensor(out=zt[0:16, :], in0=zt[0:16, :],
                                        in1=cr[:], op=ALU.mult)
                nc.sync.dma_start(zt[16:32, :], statsT[:, :])
                fw1s = hpool.tile([32, 32], f32, tag="fw1")
                nc.sync.dma_start(fw1s[:], fw1[:, :])
                fb1s = hpool.tile([32, 1], f32, tag="fb1")
                nc.sync.dma_start(fb1s[:], fb1[:, :])
                fw2s = hpool.tile([32, 16], f32, tag="fw2")
                nc.sync.dma_start(fw2s[:], fw2[:, :])
                fb2s = hpool.tile([16, 1], f32, tag="fb2")
                nc.sync.dma_start(fb2s[:], fb2[:, :])
                fw3s = hpool.tile([16, 1], f32, tag="fw3")
                nc.sync.dma_start(fw3s[:], fw3[:, :])
                fb3s = hpool.tile([1, 1], f32, tag="fb3")
                nc.sync.dma_start(fb3s[:], fb3[:, :])

                mp1 = mpool.tile([32, G], f32, tag="mp")
                nc.tensor.matmul(mp1[:], fw1s[:], zt[:], start=True, stop=True)
                h1 = hpool.tile([32, G], f32, tag="h1")
                nc.scalar.activation(h1[:], mp1[:], ACT.Relu, bias=fb1s[:, 0:1])
                mp2 = mpool.tile([16, G], f32, tag="mp")
                nc.tensor.matmul(mp2[:], fw2s[:], h1[:], start=True, stop=True)
                h2 = hpool.tile([16, G], f32, tag="h2")
                nc.scalar.activation(h2[:], mp2[:], ACT.Relu, bias=fb2s[:, 0:1])
                mp3 = mpool.tile([1, G], f32, tag="mp")
                nc.tensor.matmul(mp3[:], fw3s[:], h2[:], start=True, stop=True)
                ot = hpool.tile([1, G], f32, tag="ot")
                nc.vector.tensor_tensor(
                    out=ot[:], in0=mp3[:],
                    in1=fb3s[:, 0:1].to_broadcast([1, G]), op=ALU.add)
                nc.sync.dma_start(out_t[:, :], ot[:])

    nc.finalize()
    return nc


# ------------------------------------------------------------------- driver

def run_gat(x, stats, W1, a1s, a1d, b1, W2, a2s, a2d, b2, W3, a3s, a3d, b3,
            fw1, fb1, fw2, fb2, fw3, fb3, edge_index, batch,
            trace=False, _cache={}):
    from concourse.bass_utils import run_bass_kernel_spmd

    x = np.asarray(x, np.float32)
    stats = np.asarray(stats, np.float32)
    n_graphs = stats.shape[0]
    f_in = x.shape[1]
    meta = _prep(x, np.asarray(edge_index), np.asarray(batch), n_graphs)
    NC, PC, NSTAR = meta["NC"], meta["PC"], meta["NSTAR"]

    nc = _build(meta, n_graphs, f_in)

    # host-side input prep
    inv_pi = meta["inv_pi"]
    xs = np.zeros((NSTAR, f_in), np.float32)
    xs[:x.shape[0]] = x
    xT = np.ascontiguousarray(xs[inv_pi].T).astype(BF16)

    cntrep = np.tile((1.0 / meta["counts"]).astype(np.float32)[None, :],
                     (16, 1))
    in_common = dict(
        w1=_augment_w(np.asarray(W1, np.float32), np.asarray(a1s, np.float32),
                      np.asarray(a1d, np.float32)),
        w2=_augment_w(np.asarray(W2, np.float32), np.asarray(a2s, np.float32),
                      np.asarray(a2d, np.float32)),
        w3=_augment_w(np.asarray(W3, np.float32), np.asarray(a3s, np.float32),
                      np.asarray(a3d, np.float32)),
        b1r=np.tile(np.asarray(b1, np.float32)[None, :], (128, 1)),
        b2r=np.tile(np.asarray(b2, np.float32)[None, :], (128, 1)),
        b3r=np.tile(np.asarray(b3, np.float32)[None, :], (128, 1)),
        cntr=cntrep.astype(np.float32),
        statsT=np.ascontiguousarray(stats.T).astype(np.float32),
        fw1=np.asarray(fw1, np.float32),
        fb1=np.asarray(fb1, np.float32).reshape(32, 1),
        fw2=np.asarray(fw2, np.float32),
        fb2=np.asarray(fb2, np.float32).reshape(16, 1),
        fw3=np.asarray(fw3, np.float32),
        fb3=np.asarray(fb3, np.float32).reshape(1, 1),
        dumr=_dummy_row(),
    )
    in_maps = []
    for c in range(NC):
        m = dict(in_common)
        m["xTl"] = np.ascontiguousarray(xT[:, c * PC:(c + 1) * PC])
        m["idx"] = np.ascontiguousarray(meta["idx_all"][c])
        m["goh"] = meta["goh"][c].astype(BF16)
        in_maps.append(m)

    res = run_bass_kernel_spmd(nc, in_maps, list(range(NC)), trace=trace)
    out = res.results[0]["out"]                      # [1, G]
    return np.ascontiguousarray(out.T).astype(np.float32), res


def kernel(**inputs):
    out, _ = run_gat(**inputs)
    return out


# revision 14
# speedup vs baseline: 2.3737x; 1.1310x over previous
"""GAT (3-layer, PyG-style) forward on 8 Trainium2 NeuronCores.

Strategy (v2):
  - Node space padded to 8*PC nodes; core c owns nodes [c*PC, (c+1)*PC).
  - Per layer, a "table" in DRAM holds one 256B row per node:
      [h (64 bf16) | as (4 f32) | ad (4 f32) | pad].
  - Node phase: each core computes ONLY ITS OWN block's rows (layer 0 from
    its x slice; layers 1-2 fused into the edge-phase epilogue), then an
    AllGather collective replicates the full table to every core.
  - Edge phase (per core, per 128-node tile): dma_gather the rows of all
    in-neighbours (4 int16-addressable windows of the table), compute
    e = as_src + ad_dst on DVE (head-major compaction), leakyrelu + exp on
    the Scalar engine (Lrelu/Exp, one act table), denom via contiguous
    reduce, U = sum ex*h via an all-bf16 halving tree; self-loop handled
    densely from the node's own row.
  - out = U/denom (mean over heads for concat=False) + bias (+elu); for
    l<2 the epilogue immediately computes the NEXT layer's augmented
    node-phase matmul (transpose + matmul) and writes next-layer table rows.
  - Final: per-tile pooling matmul into PSUM, AllReduce, MLP head on-device.

Host does: graph preprocessing (degree profiles, tile clustering by
max-window-degree with fuller-bin preference, slot/pad assignment, int16
index arrays), weight augmentation, and the final [1,64] -> [64,1] reshape.
"""

import sys

sys.path.insert(0, "/opt/trn_rl_repo")

import numpy as np
import ml_dtypes

BF16 = ml_dtypes.bfloat16

NEG_SLOPE = 0.2
DUMMY_AS = -30000.0
MAXJ_CALL = 14  # <=1792 idx per dma_gather (ring 2048 w/ 32KB scratch)
NQ = 4         # SWDGE queues


# ----------------------------------------------------------------- host prep

def _prep(x, edge_index, batch, n_graphs):
    """Graph preprocessing. Returns a dict of host arrays + structure."""
    N = x.shape[0]
    NC = 8
    PC = int(np.ceil(N / NC / 128)) * 128          # nodes per core (padded)
    NSTAR = NC * PC
    TILES = PC // 128
    BLKROWS = PC + 1                               # +1 dummy row per core blk
    WIN = 2 * BLKROWS                              # gather window (2 blocks)
    assert WIN <= 32767

    src = edge_index[0].astype(np.int64)
    dst = edge_index[1].astype(np.int64)

    core_of = np.arange(NSTAR) // PC               # orig id -> core
    win_of = (core_of // 2).astype(np.int64)       # orig id -> window

    # per-dst in-degree per window (real edges only; self-loops added densely)
    degw = np.zeros((NSTAR, 4), np.int64)
    np.add.at(degw, (dst, win_of[src]), 1)

    # --- cluster nodes into tiles (per core) by window-degree profile ----
    # Greedy: place nodes in order of descending max window-degree into the
    # bin where the sum-of-window-maxima grows least; prefer fuller bins on
    # ties (concentrates padding).  ~1.56x padding vs 2.48x for sum-ordered.
    rank_of = np.empty(NSTAR, np.int64)
    tile_K = np.zeros((NC, TILES, 4), np.int64)    # per-core per-tile max deg
    for c in range(NC):
        ids = np.arange(c * PC, (c + 1) * PC)
        prof = degw[ids].astype(np.int64)
        order0 = np.argsort(-prof.max(axis=1), kind="stable")
        bins_max = np.zeros((TILES, 4), np.int64)
        bins_cnt = np.zeros(TILES, np.int64)
        assign = np.empty(PC, np.int64)
        slot_in = np.empty(PC, np.int64)
        for j in order0:
            v = prof[j]
            delta = np.maximum(bins_max, v).sum(axis=1) - bins_max.sum(axis=1)
            score = delta * 1000 - bins_cnt
            score = score + (bins_cnt >= 128) * (1 << 40)
            b_ = int(np.argmin(score))
            assign[j] = b_
            slot_in[j] = bins_cnt[b_]
            bins_cnt[b_] += 1
            bins_max[b_] = np.maximum(bins_max[b_], v)
        rank_of[ids] = assign * 128 + slot_in
        tile_K[c] = bins_max

    # order each core's tiles by K-profile (descending total) so that the
    # r-th tile of each core has a similar profile across cores (SPMD
    # uniformity: round r uses K_s(r) = max over cores).
    for c in range(NC):
        tot = tile_K[c].sum(axis=1)
        t_order = np.argsort(-tot, kind="stable")
        newpos = np.empty(TILES, np.int64)
        newpos[t_order] = np.arange(TILES)
        ids = np.arange(c * PC, (c + 1) * PC)
        r = rank_of[ids]
        rank_of[ids] = newpos[r // 128] * 128 + (r % 128)
        tile_K[c] = tile_K[c][t_order]

    K_round = tile_K.max(axis=0)                   # [TILES, 4]

    # table row of node n
    table_row = core_of * BLKROWS + rank_of

    # --- slot lists --------------------------------------------------------
    dcore = core_of[dst]
    drank = rank_of[dst]
    dwin = win_of[src]
    order = np.lexsort((dwin, drank, dcore))
    src_o, dst_o = src[order], dst[order]
    dcore_o, drank_o, dwin_o = dcore[order], drank[order], dwin[order]
    loc_o = table_row[src_o] - dwin_o * WIN        # window-local row idx
    assert loc_o.min() >= 0 and loc_o.max() < WIN

    DUMMY_LOC = PC                                  # same local idx all windows
    slots = []                                      # per core: [TILES][4] arrays [128, K]
    for c in range(NC):
        core_slots = []
        for t in range(TILES):
            wslots = []
            for s in range(4):
                K = int(K_round[t, s])
                arr = np.full((128, K), DUMMY_LOC, np.int16) if K else \
                    np.zeros((128, 0), np.int16)
                wslots.append(arr)
            core_slots.append(wslots)
        slots.append(core_slots)
    # scatter edges into slots
    kfill = np.zeros((NSTAR, 4), np.int64)
    p_all = drank_o % 128
    t_all = drank_o // 128
    for i in range(len(src_o)):
        c = dcore_o[i]
        t = t_all[i]
        s = dwin_o[i]
        p = p_all[i]
        k = kfill[dst_o[i], s]
        slots[c][t][s][p, k] = loc_o[i]
        kfill[dst_o[i], s] = k + 1

    # --- pair-merged gather call structure (uniform across cores) --------
    NPAIR = (TILES + 1) // 2
    calls = []            # (pr, s, pair_joff, cj)
    pair_J = np.zeros(NPAIR, np.int64)
    tile_rng = [[None] * 4 for _ in range(TILES)]   # (pair_joff, K) per window
    for pr in range(NPAIR):
        rA, rB = 2 * pr, min(2 * pr + 1, TILES - 1)
        single = rB == rA
        joff = 0
        for s in range(4):
            KA = int(K_round[rA, s])
            KB = 0 if single else int(K_round[rB, s])
            tile_rng[rA][s] = (joff, KA)
            if not single:
                tile_rng[rB][s] = (joff + KA, KB)
            K = KA + KB
            o = 0
            while o < K:
                cj = min(MAXJ_CALL, K - o)
                calls.append((pr, s, joff + o, cj))
                o += cj
            joff += K
        pair_J[pr] = joff
    tile_J = pair_J

    # --- int16 wrapped idx arrays per core --------------------------------
    def wrap16(ix):                                 # [n] -> [128, n//16]
        a = ix.reshape(-1, 16).T
        return np.tile(a, (8, 1))

    tile_cols = []
    off = 0
    for pr in range(NPAIR):
        ncols = int(128 * pair_J[pr]) // 16
        tile_cols.append((off, ncols))
        off += ncols

    def pair_blocks(c, pr, s):
        rA, rB = 2 * pr, min(2 * pr + 1, TILES - 1)
        bA = slots[c][rA][s]
        if rB == rA:
            return bA
        return np.concatenate([bA, slots[c][rB][s]], axis=1)

    idx_cores = []
    for c in range(NC):
        parts = []
        for (pr, s, joff, cj) in calls:
            base = tile_rng[2 * pr][s][0]
            js = joff - base
            blk = pair_blocks(c, pr, s)[:, js:js + cj]   # [128, cj]
            ix = blk.T.reshape(-1).astype(np.int16)
            parts.append(wrap16(ix).astype(np.int16))
        idx_cores.append(np.concatenate(parts, axis=1)
                         if parts else np.zeros((128, 0), np.int16))
    idx_all = np.stack(idx_cores)                    # [NC, 128, TOTC]

    # per-node permutation (global pi order)
    pi_of = core_of * PC + rank_of                   # orig -> pi position
    inv_pi = np.empty(NSTAR, np.int64)
    inv_pi[pi_of] = np.arange(NSTAR)                 # pi position -> orig

    # pooling one-hot (per core, rank order) and counts
    batch_full = np.full(NSTAR, -1, np.int64)
    batch_full[:N] = batch
    goh = np.zeros((NC, PC, n_graphs), np.float32)
    for c in range(NC):
        b = batch_full[inv_pi[c * PC:(c + 1) * PC]]
        valid = b >= 0
        goh[c, np.arange(PC)[valid], b[valid]] = 1.0
    counts = np.maximum(np.bincount(batch, minlength=n_graphs), 1.0)

    return dict(
        N=N, NC=NC, PC=PC, NSTAR=NSTAR, TILES=TILES, BLKROWS=BLKROWS,
        WIN=WIN, K_round=K_round, tile_J=tile_J, calls=calls,
        tile_cols=tile_cols, idx_all=idx_all, inv_pi=inv_pi,
        goh=goh, counts=counts, DUMMY_LOC=DUMMY_LOC,
        NPAIR=NPAIR, pair_J=pair_J, tile_rng=tile_rng,
    )


def _augment_w(W, a_s, a_d, heads=4, hid=16):
    """[F, H*C] weights -> [F, 72] augmented (bf16): [W | Was | Wad]."""
    F = W.shape[0]
    Wr = W.reshape(F, heads, hid)
    was = np.einsum("fhc,hc->fh", Wr, a_s)
    wad = np.einsum("fhc,hc->fh", Wr, a_d)
    out = np.concatenate([W, was, wad], axis=1).astype(np.float32)
    return out.astype(BF16)


def _dummy_row():
    """[1, 80] bf16-typed bytes: h=0, as=DUMMY_AS (f32), ad=0 (f32)."""
    b = bytearray(160)
    asv = np.full(4, DUMMY_AS, np.float32)
    b[128:144] = asv.tobytes()
    return np.frombuffer(bytes(b), dtype=BF16).reshape(1, 80).copy()


# ------------------------------------------------------------- kernel build

def _build(meta, n_graphs, f_in, run_layers=3):
    import concourse.bass as bass
    import concourse.tile as tile
    from concourse import bacc, mybir
    from concourse.masks import make_identity

    NC, PC, TILES = meta["NC"], meta["PC"], meta["TILES"]
    BLKROWS, WIN = meta["BLKROWS"], meta["WIN"]
    TROWS = NC * BLKROWS
    K_round = meta["K_round"]
    calls, tile_cols = meta["calls"], meta["tile_cols"]
    NPAIR, pair_J, tile_rng = meta["NPAIR"], meta["pair_J"], meta["tile_rng"]
    TOTC = meta["idx_all"].shape[2]
    G = n_graphs
    f32, bf16, i16 = mybir.dt.float32, mybir.dt.bfloat16, mybir.dt.int16
    AX, ALU = mybir.AxisListType, mybir.AluOpType
    ACT = mybir.ActivationFunctionType

    nc = bacc.Bacc(None, target_bir_lowering=False, debug=False,
                   num_devices=NC, num_swdge_queues=NQ,
                   dynamic_dma_scratch_size=32768)

    # ---- I/O ----
    xTl = nc.dram_tensor("xTl", [f_in, PC], bf16, kind="ExternalInput")
    idx_in = nc.dram_tensor("idx", [128, TOTC], i16, kind="ExternalInput")
    goh_in = nc.dram_tensor("goh", [PC, G], bf16, kind="ExternalInput")
    w1 = nc.dram_tensor("w1", [f_in, 72], bf16, kind="ExternalInput")
    w2 = nc.dram_tensor("w2", [64, 72], bf16, kind="ExternalInput")
    w3 = nc.dram_tensor("w3", [16, 72], bf16, kind="ExternalInput")
    b1r = nc.dram_tensor("b1r", [128, 64], f32, kind="ExternalInput")
    b2r = nc.dram_tensor("b2r", [128, 16], f32, kind="ExternalInput")
    b3r = nc.dram_tensor("b3r", [128, 16], f32, kind="ExternalInput")
    cntr = nc.dram_tensor("cntr", [16, G], f32, kind="ExternalInput")
    statsT = nc.dram_tensor("statsT", [16, G], f32, kind="ExternalInput")
    fw1 = nc.dram_tensor("fw1", [32, 32], f32, kind="ExternalInput")
    fb1 = nc.dram_tensor("fb1", [32, 1], f32, kind="ExternalInput")
    fw2 = nc.dram_tensor("fw2", [32, 16], f32, kind="ExternalInput")
    fb2 = nc.dram_tensor("fb2", [16, 1], f32, kind="ExternalInput")
    fw3 = nc.dram_tensor("fw3", [16, 1], f32, kind="ExternalInput")
    fb3 = nc.dram_tensor("fb3", [1, 1], f32, kind="ExternalInput")
    dumr = nc.dram_tensor("dumr", [1, 80], bf16, kind="ExternalInput")
    out_t = nc.dram_tensor("out", [1, G], f32, kind="ExternalOutput")

    LIN = [f_in, 64, 16]          # node-phase input width per layer
    LOUT = [64, 16, 16]           # edge-phase output width per layer

    with tile.TileContext(nc, num_cores=NC) as tc:
        with (
            tc.tile_pool(name="dram", bufs=1, space="DRAM") as dpool,
            tc.tile_pool(name="consts", bufs=1) as cpool,
            tc.tile_pool(name="nodein", bufs=1) as npool,
            tc.tile_pool(name="psum", bufs=2, space="PSUM") as ppool,
            tc.tile_pool(name="mlpp", bufs=1, space="PSUM") as mpool,
            tc.tile_pool(name="stage", bufs=3) as spool,
            tc.tile_pool(name="gat", bufs=3) as gpool,
            tc.tile_pool(name="idxp", bufs=6) as ipool,
            tc.tile_pool(name="edge", bufs=2) as epool,
            tc.tile_pool(name="poolacc", bufs=1, space="PSUM") as papool,
            tc.tile_pool(name="head", bufs=1) as hpool,
        ):
            # own-block table inputs (local) + AllGather'd full tables
            blk_in = [dpool.tile([BLKROWS, 128], bf16, tag=f"blk{l}",
                                 name=f"blkin{l}") for l in range(3)]
            tables = [dpool.tile([TROWS, 128], bf16, tag=f"tab{l}",
                                 name=f"table{l}", addr_space="Shared")
                      for l in range(3)]
            cc_in = dpool.tile([16, G], f32, tag="ccin")
            cc_out = dpool.tile([16, G], f32, tag="ccout",
                                addr_space="Shared")

            ident = cpool.tile([128, 128], bf16)
            make_identity(nc, ident[:])
            wsb = []
            for l, wt in enumerate((w1, w2, w3)):
                t = cpool.tile([LIN[l], 72], bf16, tag=f"w{l}", name=f"wsb{l}")
                nc.sync.dma_start(t[:], wt[:, :])
                wsb.append(t)
            brep = []
            for l, bt in enumerate((b1r, b2r, b3r)):
                tf = cpool.tile([128, LOUT[l]], f32, tag=f"bf{l}")
                nc.sync.dma_start(tf[:], bt[:, :])
                t = cpool.tile([128, LOUT[l]], bf16, tag=f"b{l}",
                               name=f"bsb{l}")
                nc.vector.tensor_copy(t[:], tf[:])
                brep.append(t)
            dum_sb = cpool.tile([1, 80], bf16)
            nc.sync.dma_start(dum_sb[:], dumr[:, :])

            pid = nc.sync.partition_id()
            pool_ps = papool.tile([16, G], f32)

            qctr = [0]

            def gather_queue():
                q = qctr[0] % NQ
                qctr[0] += 1
                return q

            def elu_inplace(x_ap, w, tmps):
                """x <- elu(x); x_ap bf16 [128, w]; tmps pool.
                min(x,0) done as -relu(-x) on ACT (f32 DVE tensor_scalar is
                pathologically slow); exp fused via scale=-1."""
                t1 = tmps.tile([128, w], bf16, tag="el1")
                t2 = tmps.tile([128, w], bf16, tag="el2")
                nc.scalar.activation(t1[:], x_ap, ACT.Relu, scale=-1.0)
                nc.scalar.activation(t1[:], t1[:], ACT.Exp, scale=-1.0)
                nc.vector.tensor_scalar(out=t1[:], in0=t1[:], scalar1=-1.0,
                                        scalar2=0.0, op0=ALU.add, op1=ALU.min)
                nc.vector.tensor_scalar_max(t2[:], x_ap, 0.0)
                nc.vector.tensor_tensor(out=x_ap, in0=t1[:], in1=t2[:],
                                        op=ALU.add)

            def pack_row(st, ps):
                """Pack PSUM [128,72] f32 -> SBUF [128,80] bf16 table row."""
                nc.scalar.copy(st[:, 0:64], ps[:, 0:64])
                stf = st[:].bitcast(f32)
                nc.vector.tensor_copy(stf[:, 32:40], ps[:, 64:72])

            # ---------------- node phase: layer 0, own block only ----------
            xin = npool.tile([f_in, PC], bf16, tag="xin")
            nc.sync.dma_start(xin[:], xTl[:, :])
            for r in range(TILES):
                ps = ppool.tile([128, 72], f32, tag="nps")
                nc.tensor.matmul(ps[:], xin[:, r * 128:(r + 1) * 128],
                                 wsb[0][:], start=True, stop=True)
                st = spool.tile([128, 80], bf16, tag="nst")
                pack_row(st, ps[:])
                nc.scalar.dma_start(blk_in[0][r * 128:r * 128 + 128, 0:80],
                                    st[:])
            nc.sync.dma_start(blk_in[0][PC:PC + 1, 0:80], dum_sb[:])
            nc.gpsimd.collective_compute(
                "AllGather", mybir.AluOpType.bypass,
                replica_groups=[list(range(NC))],
                ins=[blk_in[0].opt()], outs=[tables[0].opt()])

            for l in range(run_layers):
                table = tables[l]
                W = LOUT[l]
                for pr in range(NPAIR):
                    rA = 2 * pr
                    rB = min(2 * pr + 1, TILES - 1)
                    tiles_here = [rA] if rB == rA else [rA, rB]
                    J = int(pair_J[pr])
                    coff, ncols = tile_cols[pr]
                    gat = None
                    if J > 0:
                        it = ipool.tile([128, max(ncols, 1)], i16, tag="idx")
                        nc.sync.dma_start(it[:, 0:ncols],
                                          idx_in[:, coff:coff + ncols])
                        gat = gpool.tile([128, J * 128], bf16, tag="gat")
                        g3 = gat[:].rearrange("p (j e) -> p j e", e=128)
                        ccol = 0
                        for (pr2, s_, joff, cj) in calls:
                            if pr2 != pr:
                                continue
                            n_i = 128 * cj
                            nc.gpsimd.dma_gather(
                                g3[:, joff:joff + cj, :],
                                table[s_ * WIN:(s_ + 1) * WIN, :],
                                it[:, ccol:ccol + n_i // 16],
                                n_i, n_i, 128,
                                queue_num=gather_queue(),
                                single_packet=False)
                            ccol += n_i // 16

                    for r in tiles_here:
                        rngs = [tile_rng[r][s_] for s_ in range(4)]
                        rngs = [(o, k) for (o, k) in rngs if k > 0]
                        Jt = sum(k for _, k in rngs)
                        own = epool.tile([128, 80], bf16, tag="own")
                        base = pid * BLKROWS + r * 128
                        nc.sync.dma_start(
                            own[:], table[bass.DynSlice(base, 128), 0:80])
                        ownf = own[:].bitcast(f32)
                        as_own = ownf[:, 32:36]
                        ad_own = ownf[:, 36:40]

                        if Jt > 0:
                            g3 = gat[:].rearrange("p (j e) -> p j e", e=128)
                            gf = gat[:].bitcast(f32).rearrange(
                                "p (j q) -> p j q", q=64)
                            # e-stage, head-major: e[p, h, j] = as_src + ad_dst
                            e_t = epool.tile([128, 4 * Jt], f32, tag="e")
                            e3 = e_t[:].rearrange("p (q j) -> p q j", q=4)
                            o2 = 0
                            for (o, k) in rngs:
                                nc.vector.tensor_tensor(
                                    out=e3[:, :, o2:o2 + k],
                                    in0=gf[:, o:o + k, 32:36].rearrange(
                                        "p k q -> p q k"),
                                    in1=ad_own.unsqueeze(2).to_broadcast(
                                        [128, 4, k]),
                                    op=ALU.add)
                                o2 += k
                            # exp(leaky(x)) = max(exp(x), exp(0.2x)); Lrelu
                            # lives in a different HW act table than Exp, so
                            # using it would reload the table every tile.
                            ex_t = epool.tile([128, 4 * Jt], bf16, tag="ex")
                            ex2_t = epool.tile([128, 4 * Jt], bf16, tag="ex2")
                            nc.scalar.activation(ex_t[:], e_t[:], ACT.Exp)
                            nc.scalar.activation(ex2_t[:], e_t[:], ACT.Exp,
                                                 scale=NEG_SLOPE)
                            nc.vector.tensor_tensor(out=ex_t[:], in0=ex_t[:],
                                                    in1=ex2_t[:], op=ALU.max)
                            ex3 = ex_t[:].rearrange("p (q j) -> p q j", q=4)
                            denom = epool.tile([128, 4], f32, tag="den")
                            nc.vector.tensor_reduce(
                                denom[:], ex3, AX.X, ALU.add)
                            # v[p, j, h, q16] = h_src * ex
                            v_t = gpool.tile([128, Jt * 64], bf16, tag="vt")
                            v3 = v_t[:].rearrange("p (j h q) -> p j h q",
                                                  h=4, q=16)
                            o2 = 0
                            for (o, k) in rngs:
                                nc.vector.tensor_tensor(
                                    out=v3[:, o2:o2 + k, :, :],
                                    in0=g3[:, o:o + k, 0:64].rearrange(
                                        "p j (h q) -> p j h q", q=16),
                                    in1=ex3[:, :, o2:o2 + k].rearrange(
                                        "p q k -> p k q").unsqueeze(
                                        3).to_broadcast([128, k, 4, 16]),
                                    op=ALU.mult)
                                o2 += k

                        # self contribution
                        es = epool.tile([128, 4], f32, tag="es")
                        es2 = epool.tile([128, 4], f32, tag="es2")
                        nc.vector.tensor_tensor(out=es[:], in0=as_own,
                                                in1=ad_own, op=ALU.add)
                        nc.scalar.activation(es2[:], es[:], ACT.Exp,
                                             scale=NEG_SLOPE)
                        nc.scalar.activation(es[:], es[:], ACT.Exp)
                        nc.vector.tensor_tensor(out=es[:], in0=es[:],
                                                in1=es2[:], op=ALU.max)
                        sv = epool.tile([128, 64], f32, tag="sv")
                        nc.vector.tensor_tensor(
                            out=sv[:].rearrange("p (h q) -> p h q", q=16),
                            in0=own[:, 0:64].rearrange("p (h q) -> p h q",
                                                       q=16),
                            in1=es[:].unsqueeze(2).to_broadcast([128, 4, 16]),
                            op=ALU.mult)

                        U = epool.tile([128, 64], f32, tag="U")
                        if Jt > 0:
                            cur, n, lvl = v_t, Jt, 0
                            while n > 1:
                                half, odd = n // 2, n % 2
                                nb = half + odd
                                nxt = gpool.tile([128, nb * 64], bf16,
                                                 tag=f"tr{lvl % 2}")
                                nc.vector.tensor_tensor(
                                    out=nxt[:, 0:half * 64],
                                    in0=cur[:, 0:half * 64],
                                    in1=cur[:, half * 64:2 * half * 64],
                                    op=ALU.add)
                                if odd:
                                    nc.vector.tensor_copy(
                                        nxt[:, half * 64:nb * 64],
                                        cur[:, 2 * half * 64:n * 64])
                                cur, n, lvl = nxt, nb, lvl + 1
                            nc.vector.tensor_tensor(out=U[:], in0=cur[:, 0:64],
                                                    in1=sv[:], op=ALU.add)
                            dfull = epool.tile([128, 4], f32, tag="dful")
                            nc.vector.tensor_tensor(out=dfull[:], in0=denom[:],
                                                    in1=es[:], op=ALU.add)
                        else:
                            nc.vector.tensor_copy(U[:], sv[:])
                            dfull = es

                        recip = epool.tile([128, 4], f32, tag="rec")
                        nc.vector.reciprocal(recip[:], dfull[:])
                        if l > 0:
                            # x0.25 head-mean fold; ACT Copy(scale=) instead
                            # of the slow f32 DVE tensor_scalar
                            nc.scalar.activation(recip[:], recip[:], ACT.Copy,
                                                 scale=0.25)
                        o64 = epool.tile([128, 64], bf16, tag="o64")
                        nc.vector.tensor_tensor(
                            out=o64[:].rearrange("p (h q) -> p h q", q=16),
                            in0=U[:].rearrange("p (h q) -> p h q", q=16),
                            in1=recip[:].unsqueeze(2).to_broadcast(
                                [128, 4, 16]),
                            op=ALU.mult)
                        if l == 0:
                            nc.vector.tensor_tensor(out=o64[:], in0=o64[:],
                                                    in1=brep[0][:], op=ALU.add)
                            elu_inplace(o64[:], 64, epool)
                            xnext = o64
                        else:
                            o16 = epool.tile([128, 16], bf16, tag="o16")
                            with nc.allow_low_precision(
                                    reason="4-term head mean; gate is 2e-2"):
                                nc.vector.tensor_reduce(
                                    o16[:],
                                    o64[:].rearrange("p (h q) -> p q h", q=16),
                                    AX.X, ALU.add)
                            nc.vector.tensor_tensor(out=o16[:], in0=o16[:],
                                                    in1=brep[l][:], op=ALU.add)
                            if l == 1:
                                elu_inplace(o16[:], 16, epool)
                            xnext = o16

                        if l < 2:
                            # fused next-layer node phase for own rows
                            wout = 64 if l == 0 else 16
                            pst = ppool.tile([wout, 128], bf16, tag="pst")
                            nc.tensor.transpose(out=pst[:], in_=xnext[:],
                                                identity=ident[:])
                            stt = spool.tile([wout, 128], bf16, tag="stt")
                            nc.scalar.copy(stt[:], pst[:])
                            ps2 = ppool.tile([128, 72], f32, tag="nps")
                            nc.tensor.matmul(ps2[:], stt[:], wsb[l + 1][:],
                                             start=True, stop=True)
                            st2 = spool.tile([128, 80], bf16, tag="nst2")
                            pack_row(st2, ps2[:])
                            nc.scalar.dma_start(
                                blk_in[l + 1][r * 128:r * 128 + 128, 0:80],
                                st2[:])
                        else:
                            gt = epool.tile([128, G], bf16, tag="goh")
                            nc.sync.dma_start(
                                gt[:], goh_in[r * 128:(r + 1) * 128, :])
                            nc.tensor.matmul(pool_ps[:], xnext[:], gt[:],
                                             start=(r == 0),
                                             stop=(r == TILES - 1))

                if l < 2 and run_layers > l + 1:
                    nc.sync.dma_start(blk_in[l + 1][PC:PC + 1, 0:80],
                                      dum_sb[:])
                    nc.gpsimd.collective_compute(
                        "AllGather", mybir.AluOpType.bypass,
                        replica_groups=[list(range(NC))],
                        ins=[blk_in[l + 1].opt()], outs=[tables[l + 1].opt()])

            # ---------------- pooling + MLP head ----------------
            if run_layers == 3:
                pooled = hpool.tile([16, G], f32, tag="pooled")
                nc.scalar.copy(pooled[:], pool_ps[:])
                nc.sync.dma_start(cc_in[:, :], pooled[:])
                nc.gpsimd.collective_compute(
                    "AllReduce", mybir.AluOpType.add,
                    replica_groups=[list(range(NC))],
                    ins=[cc_in.opt()], outs=[cc_out.opt()])
                zt = hpool.tile([32, G], f32, tag="zt")
                nc.sync.dma_start(zt[0:16, :], cc_out[:, :])
                cr = hpool.tile([16, G], f32, tag="cr")
                nc.sync.dma_start(cr[:], cntr[:, :])
                nc.vector.tensor_tensor(out=zt[0:16, :], in0=zt[0:16, :],
                                        in1=cr[:], op=ALU.mult)
                nc.sync.dma_start(zt[16:32, :], statsT[:, :])
                fw1s = hpool.tile([32, 32], f32, tag="fw1")
                nc.sync.dma_start(fw1s[:], fw1[:, :])
                fb1s = hpool.tile([32, 1], f32, tag="fb1")
                nc.sync.dma_start(fb1s[:], fb1[:, :])
                fw2s = hpool.tile([32, 16], f32, tag="fw2")
                nc.sync.dma_start(fw2s[:], fw2[:, :])
                fb2s = hpool.tile([16, 1], f32, tag="fb2")
                nc.sync.dma_start(fb2s[:], fb2[:, :])
                fw3s = hpool.tile([16, 1], f32, tag="fw3")
                nc.sync.dma_start(fw3s[:], fw3[:, :])
                fb3s = hpool.tile([1, 1], f32, tag="fb3")
                nc.sync.dma_start(fb3s[:], fb3[:, :])

                mp1 = mpool.tile([32, G], f32, tag="mp")
                nc.tensor.matmul(mp1[:], fw1s[:], zt[:], start=True, stop=True)
                h1 = hpool.tile([32, G], f32, tag="h1")
                nc.scalar.activation(h1[:], mp1[:], ACT.Relu, bias=fb1s[:, 0:1])
                mp2 = mpool.tile([16, G], f32, tag="mp")
                nc.tensor.matmul(mp2[:], fw2s[:], h1[:], start=True, stop=True)
                h2 = hpool.tile([16, G], f32, tag="h2")
                nc.scalar.activation(h2[:], mp2[:], ACT.Relu, bias=fb2s[:, 0:1])
                mp3 = mpool.tile([1, G], f32, tag="mp")
                nc.tensor.matmul(mp3[:], fw3s[:], h2[:], start=True, stop=True)
                ot = hpool.tile([1, G], f32, tag="ot")
                nc.vector.tensor_tensor(
                    out=ot[:], in0=mp3[:],
                    in1=fb3s[:, 0:1].to_broadcast([1, G]), op=ALU.add)
                nc.sync.dma_start(out_t[:, :], ot[:])

    nc.finalize()
    return nc


# ------------------------------------------------------------------- driver

def run_gat(x, stats, W1, a1s, a1d, b1, W2, a2s, a2d, b2, W3, a3s, a3d, b3,
            fw1, fb1, fw2, fb2, fw3, fb3, edge_index, batch,
            trace=False, _cache={}):
    from concourse.bass_utils import run_bass_kernel_spmd

    x = np.asarray(x, np.float32)
    stats = np.asarray(stats, np.float32)
    n_graphs = stats.shape[0]
    f_in = x.shape[1]
    meta = _prep(x, np.asarray(edge_index), np.asarray(batch), n_graphs)
    NC, PC, NSTAR = meta["NC"], meta["PC"], meta["NSTAR"]

    nc = _build(meta, n_graphs, f_in)

    # host-side input prep
    inv_pi = meta["inv_pi"]
    xs = np.zeros((NSTAR, f_in), np.float32)
    xs[:x.shape[0]] = x
    xT = np.ascontiguousarray(xs[inv_pi].T).astype(BF16)

    cntrep = np.tile((1.0 / meta["counts"]).astype(np.float32)[None, :],
                     (16, 1))
    in_common = dict(
        w1=_augment_w(np.asarray(W1, np.float32), np.asarray(a1s, np.float32),
                      np.asarray(a1d, np.float32)),
        w2=_augment_w(np.asarray(W2, np.float32), np.asarray(a2s, np.float32),
                      np.asarray(a2d, np.float32)),
        w3=_augment_w(np.asarray(W3, np.float32), np.asarray(a3s, np.float32),
                      np.asarray(a3d, np.float32)),
        b1r=np.tile(np.asarray(b1, np.float32)[None, :], (128, 1)),
        b2r=np.tile(np.asarray(b2, np.float32)[None, :], (128, 1)),
        b3r=np.tile(np.asarray(b3, np.float32)[None, :], (128, 1)),
        cntr=cntrep.astype(np.float32),
        statsT=np.ascontiguousarray(stats.T).astype(np.float32),
        fw1=np.asarray(fw1, np.float32),
        fb1=np.asarray(fb1, np.float32).reshape(32, 1),
        fw2=np.asarray(fw2, np.float32),
        fb2=np.asarray(fb2, np.float32).reshape(16, 1),
        fw3=np.asarray(fw3, np.float32),
        fb3=np.asarray(fb3, np.float32).reshape(1, 1),
        dumr=_dummy_row(),
    )
    in_maps = []
    for c in range(NC):
        m = dict(in_common)
        m["xTl"] = np.ascontiguousarray(xT[:, c * PC:(c + 1) * PC])
        m["idx"] = np.ascontiguousarray(meta["idx_all"][c])
        m["goh"] = meta["goh"][c].astype(BF16)
        in_maps.append(m)

    res = run_bass_kernel_spmd(nc, in_maps, list(range(NC)), trace=trace)
    out = res.results[0]["out"]                      # [1, G]
    return np.ascontiguousarray(out.T).astype(np.float32), res


def kernel(**inputs):
    out, _ = run_gat(**inputs)
    return out


# revision 22
# speedup vs baseline: 2.6588x; 1.1201x over previous
"""GAT (3-layer, PyG-style) forward on 8 Trainium2 NeuronCores.

Strategy (v2):
  - Node space padded to 8*PC nodes; core c owns nodes [c*PC, (c+1)*PC).
  - Per layer, a "table" in DRAM holds one 256B row per node:
      [h (64 bf16) | as (4 f32) | ad (4 f32) | pad].
  - Node phase: each core computes ONLY ITS OWN block's rows (layer 0 from
    its x slice; layers 1-2 fused into the edge-phase epilogue), then an
    AllGather collective replicates the full table to every core.
  - Edge phase (per core, per 128-node tile): dma_gather the rows of all
    in-neighbours (4 int16-addressable windows of the table), compute
    e = as_src + ad_dst on DVE (head-major compaction), leakyrelu + exp on
    the Scalar engine (Lrelu/Exp, one act table), denom via contiguous
    reduce, U = sum ex*h via an all-bf16 halving tree; self-loop handled
    densely from the node's own row.
  - out = U/denom (mean over heads for concat=False) + bias (+elu); for
    l<2 the epilogue immediately computes the NEXT layer's augmented
    node-phase matmul (transpose + matmul) and writes next-layer table rows.
  - Final: per-tile pooling matmul into PSUM, AllReduce, MLP head on-device.

Host does: graph preprocessing (degree profiles, tile clustering by
max-window-degree with fuller-bin preference, slot/pad assignment, int16
index arrays), weight augmentation, and the final [1,64] -> [64,1] reshape.
"""

import sys

sys.path.insert(0, "/opt/trn_rl_repo")

import numpy as np
import ml_dtypes

BF16 = ml_dtypes.bfloat16

NEG_SLOPE = 0.2
DUMMY_AS = -30000.0
MAXJ_CALL = 14  # <=1792 idx per dma_gather (ring 2048 w/ 32KB scratch)
NQ = 4         # SWDGE queues


# ----------------------------------------------------------------- host prep

def _prep(x, edge_index, batch, n_graphs):
    """Graph preprocessing. Returns a dict of host arrays + structure."""
    N = x.shape[0]
    NC = 8
    PC = int(np.ceil(N / NC / 128)) * 128          # nodes per core (padded)
    NSTAR = NC * PC
    TILES = PC // 128
    BLKROWS = PC + 1                               # +1 dummy row per core blk
    WIN = 2 * BLKROWS                              # gather window (2 blocks)
    assert WIN <= 32767

    src = edge_index[0].astype(np.int64)
    dst = edge_index[1].astype(np.int64)

    core_of = np.arange(NSTAR) // PC               # orig id -> core
    win_of = (core_of // 2).astype(np.int64)       # orig id -> window

    # per-dst in-degree per window (real edges only; self-loops added densely)
    degw = np.zeros((NSTAR, 4), np.int64)
    np.add.at(degw, (dst, win_of[src]), 1)

    # --- cluster nodes into tiles (per core) by window-degree profile ----
    # Greedy: place nodes in order of descending max window-degree into the
    # bin where the sum-of-window-maxima grows least; prefer fuller bins on
    # ties (concentrates padding).  ~1.56x padding vs 2.48x for sum-ordered.
    rank_of = np.empty(NSTAR, np.int64)
    tile_K = np.zeros((NC, TILES, 4), np.int64)    # per-core per-tile max deg
    for c in range(NC):
        ids = np.arange(c * PC, (c + 1) * PC)
        prof = degw[ids].astype(np.int64)
        order0 = np.argsort(-prof.max(axis=1), kind="stable")
        bins_max = np.zeros((TILES, 4), np.int64)
        bins_cnt = np.zeros(TILES, np.int64)
        assign = np.empty(PC, np.int64)
        slot_in = np.empty(PC, np.int64)
        for j in order0:
            v = prof[j]
            delta = np.maximum(bins_max, v).sum(axis=1) - bins_max.sum(axis=1)
            score = delta * 1000 - bins_cnt
            score = score + (bins_cnt >= 128) * (1 << 40)
            b_ = int(np.argmin(score))
            assign[j] = b_
            slot_in[j] = bins_cnt[b_]
            bins_cnt[b_] += 1
            bins_max[b_] = np.maximum(bins_max[b_], v)
        rank_of[ids] = assign * 128 + slot_in
        tile_K[c] = bins_max

    # order each core's tiles by K-profile (descending total) so that the
    # r-th tile of each core has a similar profile across cores (SPMD
    # uniformity: round r uses K_s(r) = max over cores).
    for c in range(NC):
        tot = tile_K[c].sum(axis=1)
        t_order = np.argsort(-tot, kind="stable")
        newpos = np.empty(TILES, np.int64)
        newpos[t_order] = np.arange(TILES)
        ids = np.arange(c * PC, (c + 1) * PC)
        r = rank_of[ids]
        rank_of[ids] = newpos[r // 128] * 128 + (r % 128)
        tile_K[c] = tile_K[c][t_order]

    K_round = tile_K.max(axis=0)                   # [TILES, 4]

    # table row of node n
    table_row = core_of * BLKROWS + rank_of

    # --- slot lists --------------------------------------------------------
    dcore = core_of[dst]
    drank = rank_of[dst]
    dwin = win_of[src]
    order = np.lexsort((dwin, drank, dcore))
    src_o, dst_o = src[order], dst[order]
    dcore_o, drank_o, dwin_o = dcore[order], drank[order], dwin[order]
    loc_o = table_row[src_o] - dwin_o * WIN        # window-local row idx
    assert loc_o.min() >= 0 and loc_o.max() < WIN

    DUMMY_LOC = PC                                  # same local idx all windows
    slots = []                                      # per core: [TILES][4] arrays [128, K]
    for c in range(NC):
        core_slots = []
        for t in range(TILES):
            wslots = []
            for s in range(4):
                K = int(K_round[t, s])
                arr = np.full((128, K), DUMMY_LOC, np.int16) if K else \
                    np.zeros((128, 0), np.int16)
                wslots.append(arr)
            core_slots.append(wslots)
        slots.append(core_slots)
    # scatter edges into slots
    kfill = np.zeros((NSTAR, 4), np.int64)
    p_all = drank_o % 128
    t_all = drank_o // 128
    for i in range(len(src_o)):
        c = dcore_o[i]
        t = t_all[i]
        s = dwin_o[i]
        p = p_all[i]
        k = kfill[dst_o[i], s]
        slots[c][t][s][p, k] = loc_o[i]
        kfill[dst_o[i], s] = k + 1

    # --- pair-merged gather call structure (uniform across cores) --------
    NPAIR = (TILES + 1) // 2
    calls = []            # (pr, s, pair_joff, cj)
    pair_J = np.zeros(NPAIR, np.int64)
    tile_rng = [[None] * 4 for _ in range(TILES)]   # (pair_joff, K) per window
    for pr in range(NPAIR):
        rA, rB = 2 * pr, min(2 * pr + 1, TILES - 1)
        single = rB == rA
        joff = 0
        for s in range(4):
            KA = int(K_round[rA, s])
            KB = 0 if single else int(K_round[rB, s])
            tile_rng[rA][s] = (joff, KA)
            if not single:
                tile_rng[rB][s] = (joff + KA, KB)
            K = KA + KB
            o = 0
            while o < K:
                cj = min(MAXJ_CALL, K - o)
                calls.append((pr, s, joff + o, cj))
                o += cj
            joff += K
        pair_J[pr] = joff
    tile_J = pair_J

    # --- int16 wrapped idx arrays per core --------------------------------
    def wrap16(ix):                                 # [n] -> [128, n//16]
        a = ix.reshape(-1, 16).T
        return np.tile(a, (8, 1))

    tile_cols = []
    off = 0
    for pr in range(NPAIR):
        ncols = int(128 * pair_J[pr]) // 16
        tile_cols.append((off, ncols))
        off += ncols

    def pair_blocks(c, pr, s):
        rA, rB = 2 * pr, min(2 * pr + 1, TILES - 1)
        bA = slots[c][rA][s]
        if rB == rA:
            return bA
        return np.concatenate([bA, slots[c][rB][s]], axis=1)

    idx_cores = []
    for c in range(NC):
        parts = []
        for (pr, s, joff, cj) in calls:
            base = tile_rng[2 * pr][s][0]
            js = joff - base
            blk = pair_blocks(c, pr, s)[:, js:js + cj]   # [128, cj]
            ix = blk.T.reshape(-1).astype(np.int16)
            parts.append(wrap16(ix).astype(np.int16))
        idx_cores.append(np.concatenate(parts, axis=1)
                         if parts else np.zeros((128, 0), np.int16))
    idx_all = np.stack(idx_cores)                    # [NC, 128, TOTC]

    # per-node permutation (global pi order)
    pi_of = core_of * PC + rank_of                   # orig -> pi position
    inv_pi = np.empty(NSTAR, np.int64)
    inv_pi[pi_of] = np.arange(NSTAR)                 # pi position -> orig

    # pooling one-hot (per core, rank order) and counts
    batch_full = np.full(NSTAR, -1, np.int64)
    batch_full[:N] = batch
    goh = np.zeros((NC, PC, n_graphs), np.float32)
    for c in range(NC):
        b = batch_full[inv_pi[c * PC:(c + 1) * PC]]
        valid = b >= 0
        goh[c, np.arange(PC)[valid], b[valid]] = 1.0
    counts = np.maximum(np.bincount(batch, minlength=n_graphs), 1.0)

    return dict(
        N=N, NC=NC, PC=PC, NSTAR=NSTAR, TILES=TILES, BLKROWS=BLKROWS,
        WIN=WIN, K_round=K_round, tile_J=tile_J, calls=calls,
        tile_cols=tile_cols, idx_all=idx_all, inv_pi=inv_pi,
        goh=goh, counts=counts, DUMMY_LOC=DUMMY_LOC,
        NPAIR=NPAIR, pair_J=pair_J, tile_rng=tile_rng,
    )


def _augment_w(W, a_s, a_d, heads=4, hid=16):
    """[F, H*C] weights -> [F, 72] augmented (bf16): [W | Was | Wad]."""
    F = W.shape[0]
    Wr = W.reshape(F, heads, hid)
    was = np.einsum("fhc,hc->fh", Wr, a_s)
    wad = np.einsum("fhc,hc->fh", Wr, a_d)
    out = np.concatenate([W, was, wad], axis=1).astype(np.float32)
    return out.astype(BF16)


def _dummy_row():
    """[1, 80] bf16-typed bytes: h=0, as=DUMMY_AS (f32), ad=0 (f32)."""
    b = bytearray(160)
    asv = np.full(4, DUMMY_AS, np.float32)
    b[128:144] = asv.tobytes()
    return np.frombuffer(bytes(b), dtype=BF16).reshape(1, 80).copy()


# ------------------------------------------------------------- kernel build

def _build(meta, n_graphs, f_in, run_layers=3):
    import concourse.bass as bass
    import concourse.tile as tile
    from concourse import bacc, mybir
    from concourse.masks import make_identity

    NC, PC, TILES = meta["NC"], meta["PC"], meta["TILES"]
    BLKROWS, WIN = meta["BLKROWS"], meta["WIN"]
    TROWS = NC * BLKROWS
    K_round = meta["K_round"]
    calls, tile_cols = meta["calls"], meta["tile_cols"]
    NPAIR, pair_J, tile_rng = meta["NPAIR"], meta["pair_J"], meta["tile_rng"]
    TOTC = meta["idx_all"].shape[2]
    G = n_graphs
    f32, bf16, i16 = mybir.dt.float32, mybir.dt.bfloat16, mybir.dt.int16
    AX, ALU = mybir.AxisListType, mybir.AluOpType
    ACT = mybir.ActivationFunctionType

    nc = bacc.Bacc(None, target_bir_lowering=False, debug=False,
                   num_devices=NC, num_swdge_queues=NQ,
                   dynamic_dma_scratch_size=32768)

    # ---- I/O ----
    xTl = nc.dram_tensor("xTl", [f_in, PC], bf16, kind="ExternalInput")
    idx_in = nc.dram_tensor("idx", [128, TOTC], i16, kind="ExternalInput")
    goh_in = nc.dram_tensor("goh", [PC, G], bf16, kind="ExternalInput")
    w1 = nc.dram_tensor("w1", [f_in, 72], bf16, kind="ExternalInput")
    w2 = nc.dram_tensor("w2", [64, 72], bf16, kind="ExternalInput")
    w3 = nc.dram_tensor("w3", [16, 72], bf16, kind="ExternalInput")
    b1r = nc.dram_tensor("b1r", [128, 64], f32, kind="ExternalInput")
    b2r = nc.dram_tensor("b2r", [128, 16], f32, kind="ExternalInput")
    b3r = nc.dram_tensor("b3r", [128, 16], f32, kind="ExternalInput")
    cntr = nc.dram_tensor("cntr", [16, G], f32, kind="ExternalInput")
    statsT = nc.dram_tensor("statsT", [16, G], f32, kind="ExternalInput")
    fw1 = nc.dram_tensor("fw1", [32, 32], f32, kind="ExternalInput")
    fb1 = nc.dram_tensor("fb1", [32, 1], f32, kind="ExternalInput")
    fw2 = nc.dram_tensor("fw2", [32, 16], f32, kind="ExternalInput")
    fb2 = nc.dram_tensor("fb2", [16, 1], f32, kind="ExternalInput")
    fw3 = nc.dram_tensor("fw3", [16, 1], f32, kind="ExternalInput")
    fb3 = nc.dram_tensor("fb3", [1, 1], f32, kind="ExternalInput")
    dumr = nc.dram_tensor("dumr", [1, 80], bf16, kind="ExternalInput")
    out_t = nc.dram_tensor("out", [1, G], f32, kind="ExternalOutput")

    LIN = [f_in, 64, 16]          # node-phase input width per layer
    LOUT = [64, 16, 16]           # edge-phase output width per layer

    with tile.TileContext(nc, num_cores=NC) as tc:
        with (
            tc.tile_pool(name="dram", bufs=1, space="DRAM") as dpool,
            tc.tile_pool(name="consts", bufs=1) as cpool,
            tc.tile_pool(name="nodein", bufs=1) as npool,
            tc.tile_pool(name="psum", bufs=2, space="PSUM") as ppool,
            tc.tile_pool(name="mlpp", bufs=1, space="PSUM") as mpool,
            tc.tile_pool(name="stage", bufs=4) as spool,
            tc.tile_pool(name="gat", bufs=3) as gpool,
            tc.tile_pool(name="idxp", bufs=4) as ipool,
            tc.tile_pool(name="edge", bufs=4) as epool,
            tc.tile_pool(name="poolacc", bufs=1, space="PSUM") as papool,
            tc.tile_pool(name="head", bufs=1) as hpool,
        ):
            # own-block table inputs (local) + AllGather'd full tables
            blk_in = [dpool.tile([BLKROWS, 128], bf16, tag=f"blk{l}",
                                 name=f"blkin{l}") for l in range(3)]
            tables = [dpool.tile([TROWS, 128], bf16, tag=f"tab{l}",
                                 name=f"table{l}", addr_space="Shared")
                      for l in range(3)]
            cc_in = dpool.tile([16, G], f32, tag="ccin")
            cc_out = dpool.tile([16, G], f32, tag="ccout",
                                addr_space="Shared")

            ident = cpool.tile([128, 128], bf16)
            make_identity(nc, ident[:])
            wsb = []
            for l, wt in enumerate((w1, w2, w3)):
                t = cpool.tile([LIN[l], 72], bf16, tag=f"w{l}", name=f"wsb{l}")
                nc.sync.dma_start(t[:], wt[:, :])
                wsb.append(t)
            brep = []
            for l, bt in enumerate((b1r, b2r, b3r)):
                tf = cpool.tile([128, LOUT[l]], f32, tag=f"bf{l}")
                nc.sync.dma_start(tf[:], bt[:, :])
                t = cpool.tile([128, LOUT[l]], bf16, tag=f"b{l}",
                               name=f"bsb{l}")
                nc.vector.tensor_copy(t[:], tf[:])
                brep.append(t)
            dum_sb = cpool.tile([1, 80], bf16)
            nc.sync.dma_start(dum_sb[:], dumr[:, :])

            pid = nc.sync.partition_id()
            pool_ps = papool.tile([16, G], f32)

            qctr = [0]

            def gather_queue():
                q = qctr[0] % NQ
                qctr[0] += 1
                return q

            def elu_inplace(x_ap, w, tmps):
                """x <- elu(x); x_ap bf16 [128, w]; tmps pool.
                min(x,0) done as -relu(-x) on ACT (f32 DVE tensor_scalar is
                pathologically slow); exp fused via scale=-1."""
                t1 = tmps.tile([128, w], bf16, tag="el1")
                t2 = tmps.tile([128, w], bf16, tag="el2")
                nc.scalar.activation(t1[:], x_ap, ACT.Relu, scale=-1.0)
                nc.scalar.activation(t1[:], t1[:], ACT.Exp, scale=-1.0)
                nc.vector.tensor_scalar(out=t1[:], in0=t1[:], scalar1=-1.0,
                                        scalar2=0.0, op0=ALU.add, op1=ALU.min)
                nc.scalar.activation(t2[:], x_ap, ACT.Relu)
                nc.vector.tensor_tensor(out=x_ap, in0=t1[:], in1=t2[:],
                                        op=ALU.add)

            def pack_row(st, ps):
                """Pack PSUM [128,72] f32 -> SBUF [128,80] bf16 table row."""
                nc.scalar.copy(st[:, 0:64], ps[:, 0:64])
                stf = st[:].bitcast(f32)
                nc.vector.tensor_copy(stf[:, 32:40], ps[:, 64:72])

            # ---------------- node phase: layer 0, own block only ----------
            # x loaded in two halves to halve the pool footprint
            HTIL = (TILES + 1) // 2
            xhalf = [npool.tile([f_in, HTIL * 128], bf16, tag=f"xin{i}",
                                name=f"xin{i}")
                     for i in range(2)]
            nc.sync.dma_start(xhalf[0][:], xTl[:, 0:HTIL * 128])
            nc.sync.dma_start(xhalf[1][:, 0:PC - HTIL * 128],
                              xTl[:, HTIL * 128:PC])
            for r in range(TILES):
                xin = xhalf[r // HTIL]
                rr = r % HTIL
                ps = ppool.tile([128, 72], f32, tag="nps")
                nc.tensor.matmul(ps[:], xin[:, rr * 128:(rr + 1) * 128],
                                 wsb[0][:], start=True, stop=True)
                st = spool.tile([128, 80], bf16, tag="nst")
                pack_row(st, ps[:])
                nc.scalar.dma_start(blk_in[0][r * 128:r * 128 + 128, 0:80],
                                    st[:])
            nc.sync.dma_start(blk_in[0][PC:PC + 1, 0:80], dum_sb[:])
            nc.gpsimd.collective_compute(
                "AllGather", mybir.AluOpType.bypass,
                replica_groups=[list(range(NC))],
                ins=[blk_in[0].opt()], outs=[tables[0].opt()])

            for l in range(run_layers):
                table = tables[l]
                W = LOUT[l]
                for pr in range(NPAIR):
                    rA = 2 * pr
                    rB = min(2 * pr + 1, TILES - 1)
                    tiles_here = [rA] if rB == rA else [rA, rB]
                    J = int(pair_J[pr])
                    coff, ncols = tile_cols[pr]
                    gat = None
                    if J > 0:
                        it = ipool.tile([128, max(ncols, 1)], i16, tag="idx")
                        nc.sync.dma_start(it[:, 0:ncols],
                                          idx_in[:, coff:coff + ncols])
                        gat = gpool.tile([128, J * 128], bf16, tag="gat")
                        g3 = gat[:].rearrange("p (j e) -> p j e", e=128)
                        ccol = 0
                        for (pr2, s_, joff, cj) in calls:
                            if pr2 != pr:
                                continue
                            n_i = 128 * cj
                            nc.gpsimd.dma_gather(
                                g3[:, joff:joff + cj, :],
                                table[s_ * WIN:(s_ + 1) * WIN, :],
                                it[:, ccol:ccol + n_i // 16],
                                n_i, n_i, 128,
                                queue_num=gather_queue(),
                                single_packet=False)
                            ccol += n_i // 16

                    for r in tiles_here:
                        rngs = [tile_rng[r][s_] for s_ in range(4)]
                        rngs = [(o, k) for (o, k) in rngs if k > 0]
                        Jt = sum(k for _, k in rngs)
                        own = epool.tile([128, 80], bf16, tag="own")
                        base = pid * BLKROWS + r * 128
                        nc.sync.dma_start(
                            own[:], table[bass.DynSlice(base, 128), 0:80])
                        ownf = own[:].bitcast(f32)
                        as_own = ownf[:, 32:36]
                        ad_own = ownf[:, 36:40]

                        if Jt > 0:
                            g3 = gat[:].rearrange("p (j e) -> p j e", e=128)
                            gf = gat[:].bitcast(f32).rearrange(
                                "p (j q) -> p j q", q=64)
                            # e-stage, head-major: e[p, h, j] = as_src + ad_dst
                            e_t = epool.tile([128, 4 * Jt], f32, tag="e")
                            e3 = e_t[:].rearrange("p (q j) -> p q j", q=4)
                            o2 = 0
                            for (o, k) in rngs:
                                nc.vector.tensor_tensor(
                                    out=e3[:, :, o2:o2 + k],
                                    in0=gf[:, o:o + k, 32:36].rearrange(
                                        "p k q -> p q k"),
                                    in1=ad_own.unsqueeze(2).to_broadcast(
                                        [128, 4, k]),
                                    op=ALU.add)
                                o2 += k
                            # exp(leaky(x)) = max(exp(x), exp(0.2x)); Lrelu
                            # lives in a different HW act table than Exp, so
                            # using it would reload the table every tile.
                            ex_t = epool.tile([128, 4 * Jt], bf16, tag="ex")
                            ex2_t = epool.tile([128, 4 * Jt], bf16, tag="ex2")
                            nc.scalar.activation(ex_t[:], e_t[:], ACT.Exp)
                            nc.scalar.activation(ex2_t[:], e_t[:], ACT.Exp,
                                                 scale=NEG_SLOPE)
                            nc.vector.tensor_tensor(out=ex_t[:], in0=ex_t[:],
                                                    in1=ex2_t[:], op=ALU.max)
                            ex3 = ex_t[:].rearrange("p (q j) -> p q j", q=4)
                            denom = epool.tile([128, 4], f32, tag="den")
                            nc.vector.tensor_reduce(
                                denom[:], ex3, AX.X, ALU.add)
                            # v[p, j, h, q16] = h_src * ex
                            v_t = gpool.tile([128, Jt * 64], bf16, tag="vt")
                            v3 = v_t[:].rearrange("p (j h q) -> p j h q",
                                                  h=4, q=16)
                            o2 = 0
                            for (o, k) in rngs:
                                nc.vector.tensor_tensor(
                                    out=v3[:, o2:o2 + k, :, :],
                                    in0=g3[:, o:o + k, 0:64].rearrange(
                                        "p j (h q) -> p j h q", q=16),
                                    in1=ex3[:, :, o2:o2 + k].rearrange(
                                        "p q k -> p k q").unsqueeze(
                                        3).to_broadcast([128, k, 4, 16]),
                                    op=ALU.mult)
                                o2 += k

                        # self contribution
                        es = epool.tile([128, 4], f32, tag="es")
                        es2 = epool.tile([128, 4], f32, tag="es2")
                        nc.vector.tensor_tensor(out=es[:], in0=as_own,
                                                in1=ad_own, op=ALU.add)
                        nc.scalar.activation(es2[:], es[:], ACT.Exp,
                                             scale=NEG_SLOPE)
                        nc.scalar.activation(es[:], es[:], ACT.Exp)
                        nc.vector.tensor_tensor(out=es[:], in0=es[:],
                                                in1=es2[:], op=ALU.max)
                        sv = epool.tile([128, 64], f32, tag="sv")
                        nc.vector.tensor_tensor(
                            out=sv[:].rearrange("p (h q) -> p h q", q=16),
                            in0=own[:, 0:64].rearrange("p (h q) -> p h q",
                                                       q=16),
                            in1=es[:].unsqueeze(2).to_broadcast([128, 4, 16]),
                            op=ALU.mult)

                        U = epool.tile([128, 64], f32, tag="U")
                        if Jt > 0:
                            cur, n, lvl = v_t, Jt, 0
                            while n > 1:
                                half, odd = n // 2, n % 2
                                nb = half + odd
                                nxt = gpool.tile([128, nb * 64], bf16,
                                                 tag=f"tr{lvl % 2}")
                                nc.vector.tensor_tensor(
                                    out=nxt[:, 0:half * 64],
                                    in0=cur[:, 0:half * 64],
                                    in1=cur[:, half * 64:2 * half * 64],
                                    op=ALU.add)
                                if odd:
                                    nc.vector.tensor_copy(
                                        nxt[:, half * 64:nb * 64],
                                        cur[:, 2 * half * 64:n * 64])
                                cur, n, lvl = nxt, nb, lvl + 1
                            nc.vector.tensor_tensor(out=U[:], in0=cur[:, 0:64],
                                                    in1=sv[:], op=ALU.add)
                            dfull = epool.tile([128, 4], f32, tag="dful")
                            nc.vector.tensor_tensor(out=dfull[:], in0=denom[:],
                                                    in1=es[:], op=ALU.add)
                        else:
                            nc.vector.tensor_copy(U[:], sv[:])
                            dfull = es

                        recip = epool.tile([128, 4], f32, tag="rec")
                        nc.vector.reciprocal(recip[:], dfull[:])
                        if l > 0:
                            # x0.25 head-mean fold; ACT Copy(scale=) instead
                            # of the slow f32 DVE tensor_scalar
                            nc.scalar.activation(recip[:], recip[:], ACT.Copy,
                                                 scale=0.25)
                        o64 = epool.tile([128, 64], bf16, tag="o64")
                        nc.vector.tensor_tensor(
                            out=o64[:].rearrange("p (h q) -> p h q", q=16),
                            in0=U[:].rearrange("p (h q) -> p h q", q=16),
                            in1=recip[:].unsqueeze(2).to_broadcast(
                                [128, 4, 16]),
                            op=ALU.mult)
                        if l == 0:
                            nc.vector.tensor_tensor(out=o64[:], in0=o64[:],
                                                    in1=brep[0][:], op=ALU.add)
                            elu_inplace(o64[:], 64, epool)
                            xnext = o64
                        else:
                            o16 = epool.tile([128, 16], bf16, tag="o16")
                            with nc.allow_low_precision(
                                    reason="4-term head mean; gate is 2e-2"):
                                nc.vector.tensor_reduce(
                                    o16[:],
                                    o64[:].rearrange("p (h q) -> p q h", q=16),
                                    AX.X, ALU.add)
                            nc.vector.tensor_tensor(out=o16[:], in0=o16[:],
                                                    in1=brep[l][:], op=ALU.add)
                            if l == 1:
                                elu_inplace(o16[:], 16, epool)
                            xnext = o16

                        if l < 2:
                            # fused next-layer node phase for own rows
                            wout = 64 if l == 0 else 16
                            pst = ppool.tile([wout, 128], bf16, tag="pst")
                            nc.tensor.transpose(out=pst[:], in_=xnext[:],
                                                identity=ident[:])
                            stt = spool.tile([wout, 128], bf16, tag="stt")
                            nc.scalar.copy(stt[:], pst[:])
                            ps2 = ppool.tile([128, 72], f32, tag="nps")
                            nc.tensor.matmul(ps2[:], stt[:], wsb[l + 1][:],
                                             start=True, stop=True)
                            st2 = spool.tile([128, 80], bf16, tag="nst2")
                            pack_row(st2, ps2[:])
                            nc.scalar.dma_start(
                                blk_in[l + 1][r * 128:r * 128 + 128, 0:80],
                                st2[:])
                        else:
                            gt = epool.tile([128, G], bf16, tag="goh")
                            nc.scalar.dma_start(
                                gt[:], goh_in[r * 128:(r + 1) * 128, :])
                            nc.tensor.matmul(pool_ps[:], xnext[:], gt[:],
                                             start=(r == 0),
                                             stop=(r == TILES - 1))

                if l < 2 and run_layers > l + 1:
                    nc.sync.dma_start(blk_in[l + 1][PC:PC + 1, 0:80],
                                      dum_sb[:])
                    nc.gpsimd.collective_compute(
                        "AllGather", mybir.AluOpType.bypass,
                        replica_groups=[list(range(NC))],
                        ins=[blk_in[l + 1].opt()], outs=[tables[l + 1].opt()])

            # ---------------- pooling + MLP head ----------------
            if run_layers == 3:
                pooled = hpool.tile([16, G], f32, tag="pooled")
                nc.scalar.copy(pooled[:], pool_ps[:])
                nc.sync.dma_start(cc_in[:, :], pooled[:])
                nc.gpsimd.collective_compute(
                    "AllReduce", mybir.AluOpType.add,
                    replica_groups=[list(range(NC))],
                    ins=[cc_in.opt()], outs=[cc_out.opt()])
                zt = hpool.tile([32, G], f32, tag="zt")
                nc.sync.dma_start(zt[0:16, :], cc_out[:, :])
                cr = hpool.tile([16, G], f32, tag="cr")
                nc.sync.dma_start(cr[:], cntr[:, :])
                nc.vector.tensor_tensor(out=zt[0:16, :], in0=zt[0:16, :],
                                        in1=cr[:], op=ALU.mult)
                nc.sync.dma_start(zt[16:32, :], statsT[:, :])
                fw1s = hpool.tile([32, 32], f32, tag="fw1")
                nc.sync.dma_start(fw1s[:], fw1[:, :])
                fb1s = hpool.tile([32, 1], f32, tag="fb1")
                nc.sync.dma_start(fb1s[:], fb1[:, :])
                fw2s = hpool.tile([32, 16], f32, tag="fw2")
                nc.sync.dma_start(fw2s[:], fw2[:, :])
                fb2s = hpool.tile([16, 1], f32, tag="fb2")
                nc.sync.dma_start(fb2s[:], fb2[:, :])
                fw3s = hpool.tile([16, 1], f32, tag="fw3")
                nc.sync.dma_start(fw3s[:], fw3[:, :])
                fb3s = hpool.tile([1, 1], f32, tag="fb3")
                nc.sync.dma_start(fb3s[:], fb3[:, :])

                mp1 = mpool.tile([32, G], f32, tag="mp")
                nc.tensor.matmul(mp1[:], fw1s[:], zt[:], start=True, stop=True)
                h1 = hpool.tile([32, G], f32, tag="h1")
                nc.scalar.activation(h1[:], mp1[:], ACT.Relu, bias=fb1s[:, 0:1])
                mp2 = mpool.tile([16, G], f32, tag="mp")
                nc.tensor.matmul(mp2[:], fw2s[:], h1[:], start=True, stop=True)
                h2 = hpool.tile([16, G], f32, tag="h2")
                nc.scalar.activation(h2[:], mp2[:], ACT.Relu, bias=fb2s[:, 0:1])
                mp3 = mpool.tile([1, G], f32, tag="mp")
                nc.tensor.matmul(mp3[:], fw3s[:], h2[:], start=True, stop=True)
                ot = hpool.tile([1, G], f32, tag="ot")
                nc.vector.tensor_tensor(
                    out=ot[:], in0=mp3[:],
                    in1=fb3s[:, 0:1].to_broadcast([1, G]), op=ALU.add)
                nc.sync.dma_start(out_t[:, :], ot[:])

    nc.finalize()
    return nc


# ------------------------------------------------------------------- driver

def run_gat(x, stats, W1, a1s, a1d, b1, W2, a2s, a2d, b2, W3, a3s, a3d, b3,
            fw1, fb1, fw2, fb2, fw3, fb3, edge_index, batch,
            trace=False, _cache={}):
    from concourse.bass_utils import run_bass_kernel_spmd

    x = np.asarray(x, np.float32)
    stats = np.asarray(stats, np.float32)
    n_graphs = stats.shape[0]
    f_in = x.shape[1]
    meta = _prep(x, np.asarray(edge_index), np.asarray(batch), n_graphs)
    NC, PC, NSTAR = meta["NC"], meta["PC"], meta["NSTAR"]

    nc = _build(meta, n_graphs, f_in)

    # host-side input prep
    inv_pi = meta["inv_pi"]
    xs = np.zeros((NSTAR, f_in), np.float32)
    xs[:x.shape[0]] = x
    xT = np.ascontiguousarray(xs[inv_pi].T).astype(BF16)

    cntrep = np.tile((1.0 / meta["counts"]).astype(np.float32)[None, :],
                     (16, 1))
    in_common = dict(
        w1=_augment_w(np.asarray(W1, np.float32), np.asarray(a1s, np.float32),
                      np.asarray(a1d, np.float32)),
        w2=_augment_w(np.asarray(W2, np.float32), np.asarray(a2s, np.float32),
                      np.asarray(a2d, np.float32)),
        w3=_augment_w(np.asarray(W3, np.float32), np.asarray(a3s, np.float32),
                      np.asarray(a3d, np.float32)),
        b1r=np.tile(np.asarray(b1, np.float32)[None, :], (128, 1)),
        b2r=np.tile(np.asarray(b2, np.float32)[None, :], (128, 1)),
        b3r=np.tile(np.asarray(b3, np.float32)[None, :], (128, 1)),
        cntr=cntrep.astype(np.float32),
        statsT=np.ascontiguousarray(stats.T).astype(np.float32),
        fw1=np.asarray(fw1, np.float32),
        fb1=np.asarray(fb1, np.float32).reshape(32, 1),
        fw2=np.asarray(fw2, np.float32),
        fb2=np.asarray(fb2, np.float32).reshape(16, 1),
        fw3=np.asarray(fw3, np.float32),
        fb3=np.asarray(fb3, np.float32).reshape(1, 1),
        dumr=_dummy_row(),
    )
    in_maps = []
    for c in range(NC):
        m = dict(in_common)
        m["xTl"] = np.ascontiguousarray(xT[:, c * PC:(c + 1) * PC])
        m["idx"] = np.ascontiguousarray(meta["idx_all"][c])
        m["goh"] = meta["goh"][c].astype(BF16)
        in_maps.append(m)

    res = run_bass_kernel_spmd(nc, in_maps, list(range(NC)), trace=trace)
    out = res.results[0]["out"]                      # [1, G]
    return np.ascontiguousarray(out.T).astype(np.float32), res


def kernel(**inputs):
    out, _ = run_gat(**inputs)
    return out
